# revision 6
# baseline (speedup 1.0000x reference)
"""Causal self-attention (B=2, T=2048, C=1024, H=16, D=64) on 8 trn2 cores.

Sharding: data-parallel over B (2) x tensor-parallel over head groups (4).
Core c handles batch b = c // 4 and heads [4g, 4g+4) with g = c % 4.
Each core computes a partial output  y_local @ Wo_local.T  of shape [T, C];
the host sums the 4 partials per batch.

Per-core kernel (bf16 matmuls, f32 PSUM):
  kpadT  [128, T] per (pair, head): head's 64 k-channels + 64 zero rows,
         written directly by the k-projection copies (no separate pad pass)
  scoresT[k, q] = kpad.T @ qT     (q needs NO padding: kpad's zero rows
                                   nullify the other head's q rows)
  attnT = exp(scoresT)            (no max subtraction: |scores| <~ 16 here)
  AV with v blocks [v0 | ones64] and [ones64 | v1]: the 64 ones columns
  replicate the softmax denominator across 64 partitions FOR FREE (matmul
  cost is N-bound; M-padding costs nothing).  Normalize entirely in
  channel-major layout: DVE reciprocal of the denominator block, one tiny
  SBUF->SBUF DMA partition shift, DVE multiply into yT.  No PE transposes,
  no DMA xbar transposes, no token-major round trip.

Perf notes:
  - matmul streaming measured at ~1.96 cols/ns (power-throttled 2.4 GHz);
    cost is N-bound, so K=128 padding is free.
  - software pipeline with AV lagging scores by TWO iterations so the PE
    never waits on the ~1.3us ACT exp of the tile it consumes.
  - projections run per-chunk interleaved (q,k,v share each arriving x
    chunk; 3 concurrent PSUM accumulators) so the start is DMA-paced, and
    the very first matmul needs only wq chunk0 + a 128KB x piece.
  - pair-1 projections/outprojs are deferred thunks drained one per
    attention iteration so dense PE work fills any exp-latency gaps.
  - outproj PSUM->SBUF copies run on DVE during attention (ACT is
    exp-saturated); the final blocks use ACT (idle at the tail).
  - output written f16 (host sums partials in f32): halves HBM writes.
  - final quarter (1,3) normalizes + out-projects per 128-col block as
    each block's accumulation completes, so the serial tail is ~3us.
"""

import sys

sys.path.insert(0, "/opt/trn_rl_repo")

import numpy as np
import ml_dtypes

import concourse.bass as bass
import concourse.tile as tile
from concourse import mybir
from concourse.bass_utils import run_bass_kernel_spmd
from concourse.masks import make_upper_triangular

BF16 = mybir.dt.bfloat16
F16 = mybir.dt.float16
F32 = mybir.dt.float32

T = 2048
C = 1024
H = 16
D = 64
NB = 2  # batch
NCORES = 8
NPAIRS = 2  # head pairs per core
KC = C // 128  # 8 contraction chunks for projections
NTB = T // 128  # 16 token blocks

_nc_cache = None


def split_waits(nc, max_waits=1):
    """This walrus build rejects instructions with more than one semaphore
    wait; move excess waits onto same-engine NOPs inserted just before."""
    for fn in nc.m.functions:
        for bb in fn.blocks:
            insts = bb.instructions
            new_list = []
            changed = False
            for inst in insts:
                si = inst.sync_info
                if si is not None and len(si.on_wait) > max_waits:
                    waits = list(si.on_wait)
                    extra, keep = waits[:-max_waits], waits[-max_waits:]
                    k = 0
                    while extra:
                        chunk, extra = extra[:max_waits], extra[max_waits:]
                        nop = mybir.InstNoOp(
                            name=f"{inst.name}-wsplit{k}", engine=inst.engine
                        )
                        nop.sync_info = mybir.SyncInfo(on_wait=chunk, on_update=[])
                        new_list.append(nop)
                        changed = True
                        k += 1
                    inst.sync_info = mybir.SyncInfo(
                        on_wait=keep, on_update=list(si.on_update)
                    )
                new_list.append(inst)
            if changed:
                bb.instructions = new_list


def build_nc():
    nc = bass.Bass()

    xT = nc.dram_tensor("xT", [KC, 128, T], BF16, kind="ExternalInput")
    wq = nc.dram_tensor("wq", [NPAIRS, 128, KC, 128], BF16, kind="ExternalInput")
    wk = nc.dram_tensor("wk", [NPAIRS, 128, KC, 128], BF16, kind="ExternalInput")
    wv = nc.dram_tensor("wv", [NPAIRS, 128, KC, 128], BF16, kind="ExternalInput")
    wo = nc.dram_tensor("wo", [NPAIRS, 128, C], BF16, kind="ExternalInput")
    # f16 output: halves HBM-write traffic; host sums partials in f32
    out = nc.dram_tensor("out", [NTB, 128, C], F16, kind="ExternalOutput")

    with tile.TileContext(nc) as tc:
        with (
            tc.tile_pool(name="const", bufs=1) as const,
            tc.tile_pool(name="persist", bufs=1) as persist,
            tc.tile_pool(name="temps", bufs=1) as temps,
            tc.tile_pool(name="attnp", bufs=6) as attnp,
            tc.tile_pool(name="normp", bufs=4) as normp,
            tc.tile_pool(name="outp", bufs=3) as outp,
            tc.tile_pool(name="flow", bufs=3, space="PSUM") as flow,
            tc.tile_pool(name="acc", bufs=2, space="PSUM") as acc,
        ):
            # Deferred-emission backlog: thunks emitted one per matmul-loop
            # iteration so latency-bound chains overlap dense matmul work.
            backlog = []

            def drain_one():
                if backlog:
                    backlog.pop(0)()

            def drain_all():
                while backlog:
                    backlog.pop(0)()

            # --- SBUF tensors ----------------------------------------------
            wq_sb, wk_sb, wv_sb, wo_sb = [], [], [], []
            for p in range(NPAIRS):
                for lst, nm in ((wq_sb, "wq"), (wk_sb, "wk"), (wv_sb, "wv")):
                    lst.append(
                        persist.tile(
                            [128, KC, 128], BF16, tag=f"{nm}{p}", name=f"{nm}{p}"
                        )
                    )
                wo_sb.append(
                    persist.tile([128, C], BF16, tag=f"wo{p}", name=f"wo{p}")
                )
            x_sb = [
                persist.tile([128, T], BF16, tag=f"x{kc}", name=f"x{kc}")
                for kc in range(KC)
            ]
            # qT unpadded (both heads stacked); kpad per head (zero rows kill
            # the other head's q rows in the scores contraction)
            q_sb, kpad, v_sb, yT_sb = [], [], [], []
            for p in range(NPAIRS):
                q_sb.append(persist.tile([128, T], BF16, tag=f"qT{p}", name=f"qT{p}"))
                kpad.append(
                    [
                        persist.tile([128, T], BF16, tag=f"kp{p}{hd}", name=f"kp{p}{hd}")
                        for hd in range(2)
                    ]
                )
                v_sb.append(
                    persist.tile([128, NTB, 256], BF16, tag=f"v{p}", name=f"v{p}")
                )
                yT_sb.append(persist.tile([128, T], BF16, tag=f"yT{p}", name=f"yT{p}"))
            vt_tmps = [
                temps.tile([128, T], BF16, tag=f"vt{p}", name=f"vt{p}")
                for p in range(NPAIRS)
            ]

            # --- input DMAs ------------------------------------------------
            # issue order ~= arrival order (the DMA engines serialize
            # globally): interleave weight chunks with x chunks to match the
            # per-chunk projection consumption; first x piece is 128KB so
            # the first matmul starts as early as possible.
            loads = [
                (wq_sb[0][:, 0:1, :], wq[0][:, 0:1, :]),
                (wk_sb[0][:, 0:1, :], wk[0][:, 0:1, :]),
                (wv_sb[0][:, 0:1, :], wv[0][:, 0:1, :]),
                (x_sb[0][:, 0:512], xT[0][:, 0:512]),
                (x_sb[0][:, 512:1024], xT[0][:, 512:1024]),
            ]
            for kc in range(1, KC):
                loads += [
                    (wq_sb[0][:, kc : kc + 1, :], wq[0][:, kc : kc + 1, :]),
                    (wk_sb[0][:, kc : kc + 1, :], wk[0][:, kc : kc + 1, :]),
                    (wv_sb[0][:, kc : kc + 1, :], wv[0][:, kc : kc + 1, :]),
                    (x_sb[kc][:, 0:1024], xT[kc][:, 0:1024]),
                ]
            for kc in range(KC):
                loads.append((x_sb[kc][:, 1024:2048], xT[kc][:, 1024:2048]))
            loads += [
                (wo_sb[0][:], wo[0]),
                (wq_sb[1][:], wq[1]),
                (wk_sb[1][:], wk[1]),
                (wv_sb[1][:], wv[1]),
                (wo_sb[1][:], wo[1]),
            ]
            qrr = [nc.sync, nc.scalar, nc.gpsimd]
            for i, (dst, src_ap) in enumerate(loads):
                qrr[i % 3].dma_start(dst, src_ap)

            # --- constants / one-time zero fills ---------------------------
            # multiplicative causal mask for diagonal blocks of attnT:
            # valid where k_local <= q_local (upper triangle incl diag)
            tri = const.tile([128, 128], BF16, tag="tri", name="tri")
            make_upper_triangular(nc, tri[:], val=1.0, diag=True)
            # kpad zero halves (data halves written by the k-proj copies)
            for p in range(NPAIRS):
                nc.gpsimd.memset(kpad[p][0][64:128, :], 0.0)
                nc.gpsimd.memset(kpad[p][1][0:64, :], 0.0)
            # v blocks: [v0 | ones] and [ones | v1] per 128-col head block;
            # the ones columns are contiguous in the middle
            for p in range(NPAIRS):
                nc.gpsimd.memset(v_sb[p][:, :, 64:192], 1.0)

            # --- projections -----------------------------------------------
            def proj_copy(p, dst_kind, ps, hh):
                cols = slice(hh * 1024, (hh + 1) * 1024)
                if dst_kind == "q":
                    nc.vector.tensor_copy(q_sb[p][:, cols], ps[:])
                elif dst_kind == "k":
                    nc.vector.tensor_copy(kpad[p][0][0:64, cols], ps[0:64, :])
                    nc.vector.tensor_copy(
                        kpad[p][1][64:128, cols], ps[64:128, :]
                    )
                else:
                    if p == 0:
                        nc.scalar.copy(vt_tmps[p][:, cols], ps[:])
                    else:
                        nc.vector.tensor_copy(vt_tmps[p][:, cols], ps[:])

            def proj_pair0():
                # pair-0 runs inline at startup, per-chunk interleaved over
                # q/k/v (3 concurrent PSUM accumulators) so consumption is
                # paced to DMA arrival and the first matmul needs only wq
                # chunk0 + a 128KB x piece.
                for hh in range(2):
                    psq = flow.tile([128, 1024], F32, tag="flow", name="psq")
                    psk = flow.tile([128, 1024], F32, tag="flow", name="psk")
                    psv = flow.tile([128, 1024], F32, tag="flow", name="psv")
                    for kc in range(KC):
                        for ps, w_sb in (
                            (psq, wq_sb[0]),
                            (psk, wk_sb[0]),
                            (psv, wv_sb[0]),
                        ):
                            for o2 in range(2):
                                nc.tensor.matmul(
                                    ps[:, o2 * 512 : (o2 + 1) * 512],
                                    w_sb[:, kc, :],
                                    x_sb[kc][
                                        :,
                                        hh * 1024
                                        + o2 * 512 : hh * 1024
                                        + (o2 + 1) * 512,
                                    ],
                                    start=(kc == 0),
                                    stop=(kc == KC - 1),
                                )
                    proj_copy(0, "q", psq, hh)
                    proj_copy(0, "k", psk, hh)
                    proj_copy(0, "v", psv, hh)

            def proj_chunk1(dst_kind, w_sb, hh):
                # pair-1 projection thunk: allocates and fully consumes its
                # PSUM tile within one thunk (safe vs the flow ring rotating
                # between thunk drains)
                ps = flow.tile([128, 1024], F32, tag="flow", name="flow")
                for kc in range(KC):
                    for o2 in range(2):
                        nc.tensor.matmul(
                            ps[:, o2 * 512 : (o2 + 1) * 512],
                            w_sb[:, kc, :],
                            x_sb[kc][
                                :,
                                hh * 1024 + o2 * 512 : hh * 1024 + (o2 + 1) * 512,
                            ],
                            start=(kc == 0),
                            stop=(kc == KC - 1),
                        )
                proj_copy(1, dst_kind, ps, hh)

            def v_tr(p, hd):
                # one big xbar transpose per head: [64, 2048] -> blocked
                # [128, 16, 64] (partition-wrapped transpose); head1 lands
                # at cols 192:256 (after the ones block)
                dst_c = 0 if hd == 0 else 192
                nc.sync.dma_start_transpose(
                    v_sb[p][:, :, dst_c : dst_c + 64],
                    vt_tmps[p][hd * 64 : (hd + 1) * 64, :],
                )

            # pair 0 inline; pair 1 deferred into pair-0's attention loop
            proj_pair0()
            v_tr(0, 0)
            v_tr(0, 1)

            def mk(f, *a):
                return lambda: f(*a)

            for dst_kind, w_sb in (
                ("q", wq_sb[1]),
                ("k", wk_sb[1]),
                ("v", wv_sb[1]),
            ):
                for hh in range(2):
                    backlog.append(mk(proj_chunk1, dst_kind, w_sb, hh))
            backlog.append(mk(v_tr, 1, 0))
            backlog.append(mk(v_tr, 1, 1))

            # --- normalization (channel-major, no transposes) --------------
            shift_q = [nc.sync, nc.gpsimd]

            def norm(p, Q, y_aug, c0, c1, sq=0):
                """Normalize y_aug columns [c0:c1) into yT_sb[p].
                y_aug[0] rows 0:64 = y0, rows 64:128 = d0 (replicated);
                y_aug[1] rows 0:64 = d1, rows 64:128 = y1."""
                qlo = Q * 512
                r = normp.tile([128, 512], F32, tag="r", name="r")
                rs = normp.tile([128, 512], F32, tag="rs", name="rs")
                nc.vector.reciprocal(
                    r[64:128, c0:c1], y_aug[0][64:128, c0:c1]
                )
                nc.vector.reciprocal(r[0:64, c0:c1], y_aug[1][0:64, c0:c1])
                shift_q[sq].dma_start(rs[0:64, c0:c1], r[64:128, c0:c1])
                shift_q[1 - sq].dma_start(rs[64:128, c0:c1], r[0:64, c0:c1])
                nc.vector.tensor_mul(
                    yT_sb[p][0:64, qlo + c0 : qlo + c1],
                    y_aug[0][0:64, c0:c1],
                    rs[0:64, c0:c1],
                )
                nc.vector.tensor_mul(
                    yT_sb[p][64:128, qlo + c0 : qlo + c1],
                    y_aug[1][64:128, c0:c1],
                    rs[64:128, c0:c1],
                )

            def outproj(tb, act=False):
                def f():
                    o_sb = outp.tile([128, C], F16, tag="osb", name="osb")
                    for nch in range(2):
                        ps = flow.tile([128, 512], F32, tag="flow", name="flow")
                        for p in range(NPAIRS):
                            nc.tensor.matmul(
                                ps[:],
                                yT_sb[p][:, tb * 128 : (tb + 1) * 128],
                                wo_sb[p][:, nch * 512 : (nch + 1) * 512],
                                start=(p == 0),
                                stop=(p == NPAIRS - 1),
                            )
                        dst = o_sb[:, nch * 512 : (nch + 1) * 512]
                        if act:
                            nc.scalar.copy(dst, ps[:])
                        else:
                            nc.vector.tensor_copy(dst, ps[:])
                    nc.gpsimd.dma_start(out[tb], o_sb[:])

                return f

            # --- attention -------------------------------------------------
            for p in range(NPAIRS):
                for Q in range(4):  # q quarters of 512
                    qlo = Q * 512
                    final = (p, Q) == (1, 3)
                    y_aug = [
                        acc.tile([128, 512], F32, tag="acc", name="acc")
                        for _ in range(2)
                    ]
                    nkb = 4 * Q + 4
                    # software pipeline: AV lags scores by TWO iterations so
                    # the PE never waits on the ACT exp of the tile it is
                    # about to consume.  Each PSUM tile holds BOTH heads'
                    # score chunk (cols 0:512 / 512:1024) so a single
                    # (strided) ACT exp covers them.
                    groups = []  # per-iteration AV work

                    def flush_one(p=p, qlo=qlo, nkb=nkb, y_aug=y_aug,
                                  groups=groups):
                        for (kb, hd, at, q0, wdt) in groups.pop(0):
                            nc.tensor.matmul(
                                y_aug[hd][:, q0 - qlo : q0 - qlo + wdt],
                                v_sb[p][:, kb, hd * 128 : (hd + 1) * 128],
                                at[:, hd * 512 : hd * 512 + wdt],
                                start=(kb == 0),
                                stop=(kb == nkb - 1),
                            )

                    for kb in range(nkb):
                        qs = max(kb * 128, qlo)  # global q start
                        wdt = qlo + 512 - qs
                        ps = flow.tile([128, 1024], F32, tag="flow", name="flow")
                        for hd in range(2):
                            nc.tensor.matmul(
                                ps[:, hd * 512 : hd * 512 + wdt],
                                kpad[p][hd][:, kb * 128 : (kb + 1) * 128],
                                q_sb[p][:, qs : qs + wdt],
                                start=True,
                                stop=True,
                            )
                        if len(groups) >= 2:
                            flush_one()
                        if final and kb >= 14:
                            # block kb-14's accumulation completed in that
                            # flush (its diagonal k-block is kb-2)
                            norm(1, 3, y_aug, (kb - 14) * 128,
                                 (kb - 13) * 128, sq=kb % 2)
                            if kb == 15:
                                outproj(12)()
                        if not final and kb == nkb - 1:
                            # cols 0:256 of this quarter completed in the
                            # flush above (their diagonal blocks are <= kb-2)
                            norm(p, Q, y_aug, 0, 256, sq=0)
                        drain_one()
                        if len(backlog) > 8:
                            drain_one()
                        at = attnp.tile([128, 1024], BF16, tag="attn",
                                        name="attn")
                        if wdt == 512:
                            nc.scalar.activation(
                                at[:], ps[:], mybir.ActivationFunctionType.Exp
                            )
                        else:
                            v2 = ps[:].rearrange("p (h w) -> p h w", h=2)
                            a2 = at[:].rearrange("p (h w) -> p h w", h=2)
                            nc.scalar.activation(
                                a2[:, :, 0:wdt],
                                v2[:, :, 0:wdt],
                                mybir.ActivationFunctionType.Exp,
                            )
                        if kb * 128 >= qlo:
                            # diagonal block: causal mask, both heads
                            for hd in range(2):
                                nc.vector.tensor_mul(
                                    at[:, hd * 512 : hd * 512 + 128],
                                    at[:, hd * 512 : hd * 512 + 128],
                                    tri[:],
                                )
                        groups.append(
                            [(kb, hd, at, qs, wdt) for hd in range(2)]
                        )
                    # drain the 2 lagged iterations
                    if final:
                        flush_one()  # kb=14
                        norm(1, 3, y_aug, 256, 384, sq=0)
                        outproj(13, act=True)()
                        flush_one()  # kb=15
                        norm(1, 3, y_aug, 384, 512, sq=1)
                        outproj(14, act=True)()
                        outproj(15, act=True)()
                    else:
                        while groups:
                            flush_one()
                        norm(p, Q, y_aug, 256, 512, sq=1)
                        if p == 1:
                            # both pairs' yT ready for this quarter's blocks
                            for tb in range(4 * Q, 4 * Q + 4):
                                backlog.append(outproj(tb))
            drain_all()

    split_waits(nc)
    nc.finalize()
    return nc


def _get_nc():
    global _nc_cache
    if _nc_cache is None:
        _nc_cache = build_nc()
    return _nc_cache


def _prep_core_inputs(x, Wq, Wk, Wv, Wo, core):
    b, g = core // 4, core % 4
    bf = ml_dtypes.bfloat16
    xT_host = np.ascontiguousarray(x[b].T).reshape(KC, 128, T).astype(bf)

    def pack_w(Wm, scale=1.0):
        # [NPAIRS, 128, KC, 128]: lhsT chunks; columns = 2 heads' channels
        outw = np.empty((NPAIRS, 128, KC, 128), dtype=bf)
        for p in range(NPAIRS):
            h0 = 4 * g + 2 * p
            blk = Wm[h0 * D : (h0 + 2) * D, :].T * scale  # [C, 128]
            outw[p] = blk.reshape(KC, 128, 128).transpose(1, 0, 2).astype(bf)
        return outw

    scale = 1.0 / np.sqrt(D)
    wo_host = np.empty((NPAIRS, 128, C), dtype=bf)
    for p in range(NPAIRS):
        h0 = 4 * g + 2 * p
        wo_host[p] = Wo[:, h0 * D : (h0 + 2) * D].T.astype(bf)
    return {
        "xT": xT_host,
        "wq": pack_w(Wq, scale),
        "wk": pack_w(Wk),
        "wv": pack_w(Wv),
        "wo": wo_host,
    }


def _ensure_ntff_hook():
    """antenv.axon_hooks is missing from this image; shim it and register
    the ctypes NTFF profile hook so trace=True captures profiles."""
    import types

    if "antenv.axon_hooks" in sys.modules:
        return
    mod = types.ModuleType("antenv.axon_hooks")
    mod._hook = None

    def set_axon_ntff_profile_hook(h):
        mod._hook = h

    def get_axon_ntff_profile_hook():
        return mod._hook

    mod.set_axon_ntff_profile_hook = set_axon_ntff_profile_hook
    mod.get_axon_ntff_profile_hook = get_axon_ntff_profile_hook
    sys.modules["antenv.axon_hooks"] = mod
    try:
        from trn_agent_boot.trn_boot import _ntff_profile_via_ctypes

        mod._hook = _ntff_profile_via_ctypes("/opt/axon/libaxon_pjrt.so")
    except Exception as e:
        print(f"ntff hook setup failed: {e}")


def kernel(x, Wq, Wk, Wv, Wo, _trace=False, _tracedir=None):
    if _trace:
        _ensure_ntff_hook()
    x = np.asarray(x, dtype=np.float32)
    Wq, Wk, Wv, Wo = (np.asarray(w, dtype=np.float32) for w in (Wq, Wk, Wv, Wo))
    nc = _get_nc()
    in_maps = [_prep_core_inputs(x, Wq, Wk, Wv, Wo, c) for c in range(NCORES)]
    res = run_bass_kernel_spmd(
        nc, in_maps, core_ids=list(range(NCORES)), trace=_trace, tmpdir=_tracedir
    )
    out = np.zeros((NB, T, C), dtype=np.float32)
    for c in range(NCORES):
        out[c // 4] += res.results[c]["out"].astype(np.float32).reshape(T, C)
    if _trace:
        kernel._last_results = res
    return out


# revision 13
# speedup vs baseline: 1.0835x; 1.0835x over previous
"""Causal self-attention (B=2, T=2048, C=1024, H=16, D=64) on 8 trn2 cores.

Sharding: data-parallel over B (2) x tensor-parallel over head groups (4).
Core c handles batch b = c // 4 and heads [4g, 4g+4) with g = c % 4.
Each core computes a partial output  y_local @ Wo_local.T  of shape [T, C];
the host sums the 4 partials per batch.

Per-core kernel (bf16 matmuls, f32 PSUM):
  kpadT  [128, T] per (pair, head): head's 64 k-channels + 64 zero rows,
         written directly by the k-projection copies (no separate pad pass)
  scoresT[k, q] = kpad.T @ qT     (q needs NO padding: kpad's zero rows
                                   nullify the other head's q rows)
  attnT = exp(scoresT)            (no max subtraction: |scores| <~ 16 here)
  AV with v blocks [v0 | ones64] and [ones64 | v1]: the 64 ones columns
  replicate the softmax denominator across 64 partitions FOR FREE (matmul
  cost is N-bound; M-padding costs nothing).  Normalize entirely in
  channel-major layout: DVE reciprocal of the denominator block, one tiny
  SBUF->SBUF DMA partition shift, DVE multiply into yT.  No PE transposes,
  no DMA xbar transposes, no token-major round trip.

Perf notes:
  - matmul streaming measured at ~1.96 cols/ns (power-throttled 2.4 GHz);
    cost is N-bound, so K=128 padding is free.
  - software pipeline with AV lagging scores by TWO iterations so the PE
    never waits on the ~1.3us ACT exp of the tile it consumes.
  - projections run per-chunk interleaved (q,k,v share each arriving x
    chunk; 3 concurrent PSUM accumulators) so the start is DMA-paced, and
    the very first matmul needs only wq chunk0 + a 128KB x piece.
  - pair-1 projections/outprojs are deferred thunks drained one per
    attention iteration so dense PE work fills any exp-latency gaps.
  - outproj PSUM->SBUF copies run on DVE during attention (ACT is
    exp-saturated); the final blocks use ACT (idle at the tail).
  - output written f16 (host sums partials in f32): halves HBM writes.
  - final quarter (1,3) normalizes + out-projects per 128-col block as
    each block's accumulation completes, so the serial tail is ~3us.
"""

import sys

sys.path.insert(0, "/opt/trn_rl_repo")

import numpy as np
import ml_dtypes

import concourse.bass as bass
import concourse.tile as tile
from concourse import mybir
from concourse.bass_utils import run_bass_kernel_spmd
from concourse.masks import make_upper_triangular

BF16 = mybir.dt.bfloat16
F16 = mybir.dt.float16
F32 = mybir.dt.float32

T = 2048
C = 1024
H = 16
D = 64
NB = 2  # batch
NCORES = 8
NPAIRS = 2  # head pairs per core
KC = C // 128  # 8 contraction chunks for projections
NTB = T // 128  # 16 token blocks

_nc_cache = None


def split_waits(nc, max_waits=1):
    """This walrus build rejects instructions with more than one semaphore
    wait; move excess waits onto same-engine NOPs inserted just before."""
    for fn in nc.m.functions:
        for bb in fn.blocks:
            insts = bb.instructions
            new_list = []
            changed = False
            for inst in insts:
                si = inst.sync_info
                if si is not None and len(si.on_wait) > max_waits:
                    waits = list(si.on_wait)
                    extra, keep = waits[:-max_waits], waits[-max_waits:]
                    k = 0
                    while extra:
                        chunk, extra = extra[:max_waits], extra[max_waits:]
                        nop = mybir.InstNoOp(
                            name=f"{inst.name}-wsplit{k}", engine=inst.engine
                        )
                        nop.sync_info = mybir.SyncInfo(on_wait=chunk, on_update=[])
                        new_list.append(nop)
                        changed = True
                        k += 1
                    inst.sync_info = mybir.SyncInfo(
                        on_wait=keep, on_update=list(si.on_update)
                    )
                new_list.append(inst)
            if changed:
                bb.instructions = new_list


def build_nc():
    nc = bass.Bass()

    xT = nc.dram_tensor("xT", [KC, 128, T], BF16, kind="ExternalInput")
    wq = nc.dram_tensor("wq", [NPAIRS, 128, KC, 128], BF16, kind="ExternalInput")
    wk = nc.dram_tensor("wk", [NPAIRS, 128, KC, 128], BF16, kind="ExternalInput")
    wv = nc.dram_tensor("wv", [NPAIRS, 128, KC, 128], BF16, kind="ExternalInput")
    wo = nc.dram_tensor("wo", [NPAIRS, 128, C], BF16, kind="ExternalInput")
    # f16 output: halves HBM-write traffic; host sums partials in f32
    out = nc.dram_tensor("out", [NTB, 128, C], F16, kind="ExternalOutput")

    with tile.TileContext(nc) as tc:
        with (
            tc.tile_pool(name="const", bufs=1) as const,
            tc.tile_pool(name="persist", bufs=1) as persist,
            tc.tile_pool(name="temps", bufs=1) as temps,
            tc.tile_pool(name="attnp", bufs=6) as attnp,
            tc.tile_pool(name="normp", bufs=4) as normp,
            tc.tile_pool(name="outp", bufs=3) as outp,
            tc.tile_pool(name="flow", bufs=3, space="PSUM") as flow,
            tc.tile_pool(name="acc", bufs=2, space="PSUM") as acc,
        ):
            # Deferred-emission backlog: thunks emitted one per matmul-loop
            # iteration so latency-bound chains overlap dense matmul work.
            backlog = []

            def drain_one():
                if backlog:
                    backlog.pop(0)()

            def drain_all():
                while backlog:
                    backlog.pop(0)()

            # --- SBUF tensors ----------------------------------------------
            wq_sb, wk_sb, wv_sb, wo_sb = [], [], [], []
            for p in range(NPAIRS):
                for lst, nm in ((wq_sb, "wq"), (wk_sb, "wk"), (wv_sb, "wv")):
                    lst.append(
                        persist.tile(
                            [128, KC, 128], BF16, tag=f"{nm}{p}", name=f"{nm}{p}"
                        )
                    )
                wo_sb.append(
                    persist.tile([128, C], BF16, tag=f"wo{p}", name=f"wo{p}")
                )
            x_sb = [
                persist.tile([128, T], BF16, tag=f"x{kc}", name=f"x{kc}")
                for kc in range(KC)
            ]
            # qT unpadded (both heads stacked); kpad per head (zero rows kill
            # the other head's q rows in the scores contraction)
            q_sb, kpad, v_sb, yT_sb = [], [], [], []
            for p in range(NPAIRS):
                q_sb.append(persist.tile([128, T], BF16, tag=f"qT{p}", name=f"qT{p}"))
                kpad.append(
                    [
                        persist.tile([128, T], BF16, tag=f"kp{p}{hd}", name=f"kp{p}{hd}")
                        for hd in range(2)
                    ]
                )
                v_sb.append(
                    persist.tile([128, NTB, 256], BF16, tag=f"v{p}", name=f"v{p}")
                )
                yT_sb.append(persist.tile([128, T], BF16, tag=f"yT{p}", name=f"yT{p}"))
            vt_tmps = [
                temps.tile([128, T], BF16, tag=f"vt{p}", name=f"vt{p}")
                for p in range(NPAIRS)
            ]

            # --- input DMAs ------------------------------------------------
            # issue order ~= arrival order (the DMA engines serialize
            # globally): interleave weight chunks with x chunks to match the
            # per-chunk projection consumption; first x piece is 128KB so
            # the first matmul starts as early as possible.
            loads = [
                (wq_sb[0][:], wq[0]),
                (x_sb[0][:, 0:512], xT[0][:, 0:512]),
                (wk_sb[0][:], wk[0]),
                (x_sb[0][:, 512:1024], xT[0][:, 512:1024]),
                (wv_sb[0][:], wv[0]),
            ]
            for kc in range(1, KC):
                loads.append((x_sb[kc][:, 0:1024], xT[kc][:, 0:1024]))
            for kc in range(KC):
                loads.append((x_sb[kc][:, 1024:2048], xT[kc][:, 1024:2048]))
            loads += [
                (wo_sb[0][:], wo[0]),
                (wq_sb[1][:], wq[1]),
                (wk_sb[1][:], wk[1]),
                (wv_sb[1][:], wv[1]),
                (wo_sb[1][:], wo[1]),
            ]
            qrr = [nc.sync, nc.scalar, nc.gpsimd]
            for i, (dst, src_ap) in enumerate(loads):
                qrr[i % 3].dma_start(dst, src_ap)

            # --- constants / one-time zero fills ---------------------------
            # multiplicative causal mask for diagonal blocks of attnT:
            # valid where k_local <= q_local (upper triangle incl diag)
            tri = const.tile([128, 128], BF16, tag="tri", name="tri")
            make_upper_triangular(nc, tri[:], val=1.0, diag=True)
            # kpad zero halves (data halves written by the k-proj copies)
            for p in range(NPAIRS):
                nc.gpsimd.memset(kpad[p][0][64:128, :], 0.0)
                nc.gpsimd.memset(kpad[p][1][0:64, :], 0.0)
            # v blocks: [v0 | ones] and [ones | v1] per 128-col head block;
            # the ones columns are contiguous in the middle
            for p in range(NPAIRS):
                nc.gpsimd.memset(v_sb[p][:, :, 64:192], 1.0)

            # --- projections -----------------------------------------------
            def proj_copy(p, dst_kind, ps, hh):
                cols = slice(hh * 1024, (hh + 1) * 1024)
                if dst_kind == "q":
                    nc.vector.tensor_copy(q_sb[p][:, cols], ps[:])
                elif dst_kind == "k":
                    nc.vector.tensor_copy(kpad[p][0][0:64, cols], ps[0:64, :])
                    nc.vector.tensor_copy(
                        kpad[p][1][64:128, cols], ps[64:128, :]
                    )
                else:
                    if p == 0:
                        nc.scalar.copy(vt_tmps[p][:, cols], ps[:])
                    else:
                        nc.vector.tensor_copy(vt_tmps[p][:, cols], ps[:])

            def proj_pair0():
                # pair-0 runs inline at startup, per-chunk interleaved over
                # q/k/v (3 concurrent PSUM accumulators) so consumption is
                # paced to DMA arrival and the first matmul needs only wq
                # chunk0 + a 128KB x piece.
                for hh in range(2):
                    psq = flow.tile([128, 1024], F32, tag="flow", name="psq")
                    psk = flow.tile([128, 1024], F32, tag="flow", name="psk")
                    psv = flow.tile([128, 1024], F32, tag="flow", name="psv")
                    for kc in range(KC):
                        for ps, w_sb in (
                            (psq, wq_sb[0]),
                            (psk, wk_sb[0]),
                            (psv, wv_sb[0]),
                        ):
                            for o2 in range(2):
                                nc.tensor.matmul(
                                    ps[:, o2 * 512 : (o2 + 1) * 512],
                                    w_sb[:, kc, :],
                                    x_sb[kc][
                                        :,
                                        hh * 1024
                                        + o2 * 512 : hh * 1024
                                        + (o2 + 1) * 512,
                                    ],
                                    start=(kc == 0),
                                    stop=(kc == KC - 1),
                                )
                    proj_copy(0, "q", psq, hh)
                    proj_copy(0, "k", psk, hh)
                    proj_copy(0, "v", psv, hh)

            def proj_chunk1(dst_kind, w_sb, hh):
                # pair-1 projection thunk: allocates and fully consumes its
                # PSUM tile within one thunk (safe vs the flow ring rotating
                # between thunk drains)
                ps = flow.tile([128, 1024], F32, tag="flow", name="flow")
                for kc in range(KC):
                    for o2 in range(2):
                        nc.tensor.matmul(
                            ps[:, o2 * 512 : (o2 + 1) * 512],
                            w_sb[:, kc, :],
                            x_sb[kc][
                                :,
                                hh * 1024 + o2 * 512 : hh * 1024 + (o2 + 1) * 512,
                            ],
                            start=(kc == 0),
                            stop=(kc == KC - 1),
                        )
                proj_copy(1, dst_kind, ps, hh)

            def v_tr(p, hd):
                # one big xbar transpose per head: [64, 2048] -> blocked
                # [128, 16, 64] (partition-wrapped transpose); head1 lands
                # at cols 192:256 (after the ones block)
                dst_c = 0 if hd == 0 else 192
                nc.sync.dma_start_transpose(
                    v_sb[p][:, :, dst_c : dst_c + 64],
                    vt_tmps[p][hd * 64 : (hd + 1) * 64, :],
                )

            # pair 0 inline; pair 1 deferred into pair-0's attention loop
            proj_pair0()
            v_tr(0, 0)
            v_tr(0, 1)

            def mk(f, *a):
                return lambda: f(*a)

            for dst_kind, w_sb in (
                ("q", wq_sb[1]),
                ("k", wk_sb[1]),
                ("v", wv_sb[1]),
            ):
                for hh in range(2):
                    backlog.append(mk(proj_chunk1, dst_kind, w_sb, hh))
            backlog.append(mk(v_tr, 1, 0))
            backlog.append(mk(v_tr, 1, 1))

            # --- normalization (channel-major, no transposes) --------------
            def norm(p, Q, y_aug, c0, c1, sq=0):
                """Normalize y_aug columns [c0:c1) into yT_sb[p].
                y_aug[0] rows 0:64 = y0, rows 64:128 = d0 (replicated);
                y_aug[1] rows 0:64 = d1, rows 64:128 = y1."""
                qlo = Q * 512
                r = normp.tile([128, 512], F32, tag="r", name="r")
                rs = normp.tile([128, 512], F32, tag="rs", name="rs")
                # ACT-table reciprocal via exp(-ln d): the DVE reciprocal is
                # ~4.3 cyc/el (1.6us per block) and gates the quarter
                # pipeline; ACT sits next to PSUM at ~1 el/cyc.  d >= 1
                # always (diagonal exp(0)=1 is in every row sum), so ln is
                # safe and exp(-ln d) is in (0, 1].
                ln0 = normp.tile([128, 512], F32, tag="lnd", name="lnd")
                nc.scalar.activation(
                    ln0[64:128, c0:c1],
                    y_aug[0][64:128, c0:c1],
                    mybir.ActivationFunctionType.Ln,
                )
                nc.scalar.activation(
                    ln0[0:64, c0:c1],
                    y_aug[1][0:64, c0:c1],
                    mybir.ActivationFunctionType.Ln,
                )
                nc.scalar.activation(
                    r[64:128, c0:c1],
                    ln0[64:128, c0:c1],
                    mybir.ActivationFunctionType.Exp,
                    scale=-1.0,
                )
                nc.scalar.activation(
                    r[0:64, c0:c1],
                    ln0[0:64, c0:c1],
                    mybir.ActivationFunctionType.Exp,
                    scale=-1.0,
                )
                nc.sync.dma_start(rs[0:64, c0:c1], r[64:128, c0:c1])
                nc.sync.dma_start(rs[64:128, c0:c1], r[0:64, c0:c1])
                nc.vector.tensor_mul(
                    yT_sb[p][0:64, qlo + c0 : qlo + c1],
                    y_aug[0][0:64, c0:c1],
                    rs[0:64, c0:c1],
                )
                nc.vector.tensor_mul(
                    yT_sb[p][64:128, qlo + c0 : qlo + c1],
                    y_aug[1][64:128, c0:c1],
                    rs[64:128, c0:c1],
                )

            def outproj(tb, act=False):
                def f():
                    o_sb = outp.tile([128, C], F16, tag="osb", name="osb")
                    for nch in range(2):
                        ps = flow.tile([128, 512], F32, tag="flow", name="flow")
                        for p in range(NPAIRS):
                            nc.tensor.matmul(
                                ps[:],
                                yT_sb[p][:, tb * 128 : (tb + 1) * 128],
                                wo_sb[p][:, nch * 512 : (nch + 1) * 512],
                                start=(p == 0),
                                stop=(p == NPAIRS - 1),
                            )
                        dst = o_sb[:, nch * 512 : (nch + 1) * 512]
                        if act:
                            # tail blocks: ACT is idle after the last exp
                            nc.scalar.copy(dst, ps[:])
                        else:
                            # attention phase: ACT is exp-saturated
                            nc.vector.tensor_copy(dst, ps[:])
                    nc.gpsimd.dma_start(out[tb], o_sb[:])

                return f

            # --- attention -------------------------------------------------
            for p in range(NPAIRS):
                for Q in range(4):  # q quarters of 512
                    qlo = Q * 512
                    final = (p, Q) == (1, 3)
                    y_aug = [
                        acc.tile([128, 512], F32, tag="acc", name="acc")
                        for _ in range(2)
                    ]
                    nkb = 4 * Q + 4
                    # software pipeline: AV lags scores by TWO iterations so
                    # the PE never waits on the ACT exp of the tile it is
                    # about to consume.  Each PSUM tile holds BOTH heads'
                    # score chunk (cols 0:512 / 512:1024) so a single
                    # (strided) ACT exp covers them.
                    groups = []  # per-iteration AV work

                    def flush_one(p=p, qlo=qlo, nkb=nkb, y_aug=y_aug,
                                  groups=groups):
                        for (kb, hd, at, q0, wdt) in groups.pop(0):
                            nc.tensor.matmul(
                                y_aug[hd][:, q0 - qlo : q0 - qlo + wdt],
                                v_sb[p][:, kb, hd * 128 : (hd + 1) * 128],
                                at[:, hd * 512 : hd * 512 + wdt],
                                start=(kb == 0),
                                stop=(kb == nkb - 1),
                            )

                    for kb in range(nkb):
                        qs = max(kb * 128, qlo)  # global q start
                        wdt = qlo + 512 - qs
                        ps = flow.tile([128, 1024], F32, tag="flow", name="flow")
                        for hd in range(2):
                            nc.tensor.matmul(
                                ps[:, hd * 512 : hd * 512 + wdt],
                                kpad[p][hd][:, kb * 128 : (kb + 1) * 128],
                                q_sb[p][:, qs : qs + wdt],
                                start=True,
                                stop=True,
                            )
                        if len(groups) >= 2:
                            flush_one()
                        if final and kb >= 14:
                            # block kb-14's accumulation completed in that
                            # flush (its diagonal k-block is kb-2)
                            norm(1, 3, y_aug, (kb - 14) * 128,
                                 (kb - 13) * 128, sq=kb % 2)
                            if kb == 15:
                                outproj(12)()
                        if not final and kb == nkb - 1:
                            # cols 0:256 of this quarter completed in the
                            # flush above (their diagonal blocks are <= kb-2)
                            norm(p, Q, y_aug, 0, 256, sq=0)
                        drain_one()
                        if len(backlog) > 8:
                            drain_one()
                        at = attnp.tile([128, 1024], BF16, tag="attn",
                                        name="attn")
                        if wdt == 512:
                            nc.scalar.activation(
                                at[:], ps[:], mybir.ActivationFunctionType.Exp
                            )
                        else:
                            v2 = ps[:].rearrange("p (h w) -> p h w", h=2)
                            a2 = at[:].rearrange("p (h w) -> p h w", h=2)
                            nc.scalar.activation(
                                a2[:, :, 0:wdt],
                                v2[:, :, 0:wdt],
                                mybir.ActivationFunctionType.Exp,
                            )
                        if kb * 128 >= qlo:
                            # diagonal block: causal mask, both heads
                            for hd in range(2):
                                nc.vector.tensor_mul(
                                    at[:, hd * 512 : hd * 512 + 128],
                                    at[:, hd * 512 : hd * 512 + 128],
                                    tri[:],
                                )
                        groups.append(
                            [(kb, hd, at, qs, wdt) for hd in range(2)]
                        )
                    # drain the 2 lagged iterations
                    if final:
                        flush_one()  # kb=14
                        norm(1, 3, y_aug, 256, 384, sq=0)
                        outproj(13, act=True)()
                        flush_one()  # kb=15
                        norm(1, 3, y_aug, 384, 512, sq=1)
                        outproj(14, act=True)()
                        outproj(15, act=True)()
                    else:
                        flush_one()  # group nkb-2
                        norm(p, Q, y_aug, 256, 384, sq=1)
                        flush_one()  # group nkb-1
                        norm(p, Q, y_aug, 384, 512, sq=0)
                        if p == 1:
                            # both pairs' yT ready for this quarter's blocks
                            for tb in range(4 * Q, 4 * Q + 4):
                                backlog.append(outproj(tb))
            drain_all()

    split_waits(nc)
    nc.finalize()
    return nc


def _get_nc():
    global _nc_cache
    if _nc_cache is None:
        _nc_cache = build_nc()
    return _nc_cache


def _prep_core_inputs(x, Wq, Wk, Wv, Wo, core):
    b, g = core // 4, core % 4
    bf = ml_dtypes.bfloat16
    xT_host = np.ascontiguousarray(x[b].T).reshape(KC, 128, T).astype(bf)

    def pack_w(Wm, scale=1.0):
        # [NPAIRS, 128, KC, 128]: lhsT chunks; columns = 2 heads' channels
        outw = np.empty((NPAIRS, 128, KC, 128), dtype=bf)
        for p in range(NPAIRS):
            h0 = 4 * g + 2 * p
            blk = Wm[h0 * D : (h0 + 2) * D, :].T * scale  # [C, 128]
            outw[p] = blk.reshape(KC, 128, 128).transpose(1, 0, 2).astype(bf)
        return outw

    scale = 1.0 / np.sqrt(D)
    wo_host = np.empty((NPAIRS, 128, C), dtype=bf)
    for p in range(NPAIRS):
        h0 = 4 * g + 2 * p
        wo_host[p] = Wo[:, h0 * D : (h0 + 2) * D].T.astype(bf)
    return {
        "xT": xT_host,
        "wq": pack_w(Wq, scale),
        "wk": pack_w(Wk),
        "wv": pack_w(Wv),
        "wo": wo_host,
    }


def _ensure_ntff_hook():
    """antenv.axon_hooks is missing from this image; shim it and register
    the ctypes NTFF profile hook so trace=True captures profiles."""
    import types

    if "antenv.axon_hooks" in sys.modules:
        return
    mod = types.ModuleType("antenv.axon_hooks")
    mod._hook = None

    def set_axon_ntff_profile_hook(h):
        mod._hook = h

    def get_axon_ntff_profile_hook():
        return mod._hook

    mod.set_axon_ntff_profile_hook = set_axon_ntff_profile_hook
    mod.get_axon_ntff_profile_hook = get_axon_ntff_profile_hook
    sys.modules["antenv.axon_hooks"] = mod
    try:
        from trn_agent_boot.trn_boot import _ntff_profile_via_ctypes

        mod._hook = _ntff_profile_via_ctypes("/opt/axon/libaxon_pjrt.so")
    except Exception as e:
        print(f"ntff hook setup failed: {e}")


def kernel(x, Wq, Wk, Wv, Wo, _trace=False, _tracedir=None):
    if _trace:
        _ensure_ntff_hook()
    x = np.asarray(x, dtype=np.float32)
    Wq, Wk, Wv, Wo = (np.asarray(w, dtype=np.float32) for w in (Wq, Wk, Wv, Wo))
    nc = _get_nc()
    in_maps = [_prep_core_inputs(x, Wq, Wk, Wv, Wo, c) for c in range(NCORES)]
    res = run_bass_kernel_spmd(
        nc, in_maps, core_ids=list(range(NCORES)), trace=_trace, tmpdir=_tracedir
    )
    out = np.zeros((NB, T, C), dtype=np.float32)
    for c in range(NCORES):
        out[c // 4] += res.results[c]["out"].astype(np.float32).reshape(T, C)
    if _trace:
        kernel._last_results = res
    return out


# revision 17
# speedup vs baseline: 1.1056x; 1.0204x over previous
"""Causal self-attention (B=2, T=2048, C=1024, H=16, D=64) on 8 trn2 cores.

Sharding: data-parallel over B (2) x tensor-parallel over head groups (4).
Core c handles batch b = c // 4 and heads [4g, 4g+4) with g = c % 4.
Each core computes a partial output  y_local @ Wo_local.T  of shape [T, C];
the host sums the 4 partials per batch.

Per-core kernel (bf16 matmuls, f32 PSUM):
  kpadT  [128, T] per (pair, head): head's 64 k-channels + 64 zero rows,
         written directly by the k-projection copies (no separate pad pass)
  scoresT[k, q] = kpad.T @ qT     (q needs NO padding: kpad's zero rows
                                   nullify the other head's q rows)
  attnT = exp(scoresT)            (no max subtraction: |scores| <~ 16 here)
  AV with v blocks [v0 | ones64] and [ones64 | v1]: the 64 ones columns
  replicate the softmax denominator across 64 partitions FOR FREE (matmul
  cost is N-bound; M-padding costs nothing).  Normalize entirely in
  channel-major layout: DVE reciprocal of the denominator block, one tiny
  SBUF->SBUF DMA partition shift, DVE multiply into yT.  No PE transposes,
  no DMA xbar transposes, no token-major round trip.

Perf notes:
  - matmul streaming measured at ~1.96 cols/ns (power-throttled 2.4 GHz);
    cost is N-bound, so K=128 padding is free.
  - software pipeline with AV lagging scores by TWO iterations so the PE
    never waits on the ~1.3us ACT exp of the tile it consumes.
  - projections run per-chunk interleaved (q,k,v share each arriving x
    chunk; 3 concurrent PSUM accumulators) so the start is DMA-paced, and
    the very first matmul needs only wq chunk0 + a 128KB x piece.
  - pair-1 projections/outprojs are deferred thunks drained one per
    attention iteration so dense PE work fills any exp-latency gaps.
  - outproj PSUM->SBUF copies run on DVE during attention (ACT is
    exp-saturated); the final blocks use ACT (idle at the tail).
  - output written f16 (host sums partials in f32): halves HBM writes.
  - final quarter (1,3) normalizes + out-projects per 128-col block as
    each block's accumulation completes, so the serial tail is ~3us.
"""

import sys

sys.path.insert(0, "/opt/trn_rl_repo")

import numpy as np
import ml_dtypes

import concourse.bass as bass
import concourse.tile as tile
from concourse import mybir
from concourse.bass_utils import run_bass_kernel_spmd
from concourse.masks import make_upper_triangular

BF16 = mybir.dt.bfloat16
F16 = mybir.dt.float16
F32 = mybir.dt.float32

T = 2048
C = 1024
H = 16
D = 64
NB = 2  # batch
NCORES = 8
NPAIRS = 2  # head pairs per core
KC = C // 128  # 8 contraction chunks for projections
NTB = T // 128  # 16 token blocks

_nc_cache = None


def split_waits(nc, max_waits=1):
    """This walrus build rejects instructions with more than one semaphore
    wait; move excess waits onto same-engine NOPs inserted just before."""
    for fn in nc.m.functions:
        for bb in fn.blocks:
            insts = bb.instructions
            new_list = []
            changed = False
            for inst in insts:
                si = inst.sync_info
                if si is not None and len(si.on_wait) > max_waits:
                    waits = list(si.on_wait)
                    extra, keep = waits[:-max_waits], waits[-max_waits:]
                    k = 0
                    while extra:
                        chunk, extra = extra[:max_waits], extra[max_waits:]
                        nop = mybir.InstNoOp(
                            name=f"{inst.name}-wsplit{k}", engine=inst.engine
                        )
                        nop.sync_info = mybir.SyncInfo(on_wait=chunk, on_update=[])
                        new_list.append(nop)
                        changed = True
                        k += 1
                    inst.sync_info = mybir.SyncInfo(
                        on_wait=keep, on_update=list(si.on_update)
                    )
                new_list.append(inst)
            if changed:
                bb.instructions = new_list


def build_nc():
    nc = bass.Bass()

    xT = nc.dram_tensor("xT", [KC, 128, T], BF16, kind="ExternalInput")
    wq = nc.dram_tensor("wq", [NPAIRS, 128, KC, 128], BF16, kind="ExternalInput")
    wk = nc.dram_tensor("wk", [NPAIRS, 128, KC, 128], BF16, kind="ExternalInput")
    wv = nc.dram_tensor("wv", [NPAIRS, 128, KC, 128], BF16, kind="ExternalInput")
    wo = nc.dram_tensor("wo", [NPAIRS, 128, C], BF16, kind="ExternalInput")
    # f16 output: halves HBM-write traffic; host sums partials in f32
    out = nc.dram_tensor("out", [NTB, 128, C], F16, kind="ExternalOutput")

    with tile.TileContext(nc) as tc:
        with (
            tc.tile_pool(name="const", bufs=1) as const,
            tc.tile_pool(name="persist", bufs=1) as persist,
            tc.tile_pool(name="temps", bufs=1) as temps,
            tc.tile_pool(name="attnp", bufs=6) as attnp,
            tc.tile_pool(name="normp", bufs=4) as normp,
            tc.tile_pool(name="outp", bufs=3) as outp,
            tc.tile_pool(name="flow", bufs=3, space="PSUM") as flow,
            tc.tile_pool(name="acc", bufs=2, space="PSUM") as acc,
        ):
            # Deferred-emission backlog: thunks emitted one per matmul-loop
            # iteration so latency-bound chains overlap dense matmul work.
            backlog = []

            def drain_one():
                if backlog:
                    backlog.pop(0)()

            def drain_all():
                while backlog:
                    backlog.pop(0)()

            # --- SBUF tensors ----------------------------------------------
            wq_sb, wk_sb, wv_sb, wo_sb = [], [], [], []
            for p in range(NPAIRS):
                for lst, nm in ((wq_sb, "wq"), (wk_sb, "wk"), (wv_sb, "wv")):
                    lst.append(
                        persist.tile(
                            [128, KC, 128], BF16, tag=f"{nm}{p}", name=f"{nm}{p}"
                        )
                    )
                wo_sb.append(
                    persist.tile([128, C], BF16, tag=f"wo{p}", name=f"wo{p}")
                )
            x_sb = [
                persist.tile([128, T], BF16, tag=f"x{kc}", name=f"x{kc}")
                for kc in range(KC)
            ]
            # qT unpadded (both heads stacked); kpad per head (zero rows kill
            # the other head's q rows in the scores contraction)
            q_sb, kpad, v_sb, yT_sb = [], [], [], []
            for p in range(NPAIRS):
                q_sb.append(persist.tile([128, T], BF16, tag=f"qT{p}", name=f"qT{p}"))
                kpad.append(
                    [
                        persist.tile([128, T], BF16, tag=f"kp{p}{hd}", name=f"kp{p}{hd}")
                        for hd in range(2)
                    ]
                )
                v_sb.append(
                    persist.tile([128, NTB, 256], BF16, tag=f"v{p}", name=f"v{p}")
                )
                yT_sb.append(persist.tile([128, T], BF16, tag=f"yT{p}", name=f"yT{p}"))
            vt_tmps = [
                temps.tile([128, T], BF16, tag=f"vt{p}", name=f"vt{p}")
                for p in range(NPAIRS)
            ]

            # --- input DMAs ------------------------------------------------
            # issue order ~= arrival order (the DMA engines serialize
            # globally): interleave weight chunks with x chunks to match the
            # per-chunk projection consumption; first x piece is 128KB so
            # the first matmul starts as early as possible.
            loads = [
                (wq_sb[0][:], wq[0]),
                (x_sb[0][:, 0:512], xT[0][:, 0:512]),
                (wk_sb[0][:], wk[0]),
                (x_sb[0][:, 512:1024], xT[0][:, 512:1024]),
                (wv_sb[0][:], wv[0]),
            ]
            for kc in range(1, KC):
                loads.append((x_sb[kc][:, 0:1024], xT[kc][:, 0:1024]))
            for kc in range(KC):
                loads.append((x_sb[kc][:, 1024:2048], xT[kc][:, 1024:2048]))
            loads += [
                (wo_sb[0][:], wo[0]),
                (wq_sb[1][:], wq[1]),
                (wk_sb[1][:], wk[1]),
                (wv_sb[1][:], wv[1]),
                (wo_sb[1][:], wo[1]),
            ]
            qrr = [nc.sync, nc.scalar, nc.gpsimd]
            for i, (dst, src_ap) in enumerate(loads):
                qrr[i % 3].dma_start(dst, src_ap)

            # --- constants / one-time zero fills ---------------------------
            # multiplicative causal mask for diagonal blocks of attnT:
            # valid where k_local <= q_local (upper triangle incl diag)
            tri = const.tile([128, 128], BF16, tag="tri", name="tri")
            make_upper_triangular(nc, tri[:], val=1.0, diag=True)
            # kpad zero halves (data halves written by the k-proj copies)
            for p in range(NPAIRS):
                nc.gpsimd.memset(kpad[p][0][64:128, :], 0.0)
                nc.gpsimd.memset(kpad[p][1][0:64, :], 0.0)
            # v blocks: [v0 | ones] and [ones | v1] per 128-col head block;
            # the ones columns are contiguous in the middle
            for p in range(NPAIRS):
                nc.gpsimd.memset(v_sb[p][:, :, 64:192], 1.0)

            # --- projections -----------------------------------------------
            def proj_copy(p, dst_kind, ps, hh):
                cols = slice(hh * 1024, (hh + 1) * 1024)
                if dst_kind == "q":
                    nc.vector.tensor_copy(q_sb[p][:, cols], ps[:])
                elif dst_kind == "k":
                    nc.vector.tensor_copy(kpad[p][0][0:64, cols], ps[0:64, :])
                    nc.vector.tensor_copy(
                        kpad[p][1][64:128, cols], ps[64:128, :]
                    )
                else:
                    if p == 0:
                        nc.scalar.copy(vt_tmps[p][:, cols], ps[:])
                    else:
                        nc.vector.tensor_copy(vt_tmps[p][:, cols], ps[:])

            def proj_pair0():
                # pair-0 runs inline at startup, per-chunk interleaved over
                # q/k/v (3 concurrent PSUM accumulators) so consumption is
                # paced to DMA arrival and the first matmul needs only wq
                # chunk0 + a 128KB x piece.
                for hh in range(2):
                    psq = flow.tile([128, 1024], F32, tag="flow", name="psq")
                    psk = flow.tile([128, 1024], F32, tag="flow", name="psk")
                    psv = flow.tile([128, 1024], F32, tag="flow", name="psv")
                    for kc in range(KC):
                        for ps, w_sb in (
                            (psq, wq_sb[0]),
                            (psk, wk_sb[0]),
                            (psv, wv_sb[0]),
                        ):
                            for o2 in range(2):
                                nc.tensor.matmul(
                                    ps[:, o2 * 512 : (o2 + 1) * 512],
                                    w_sb[:, kc, :],
                                    x_sb[kc][
                                        :,
                                        hh * 1024
                                        + o2 * 512 : hh * 1024
                                        + (o2 + 1) * 512,
                                    ],
                                    start=(kc == 0),
                                    stop=(kc == KC - 1),
                                )
                    proj_copy(0, "q", psq, hh)
                    proj_copy(0, "k", psk, hh)
                    proj_copy(0, "v", psv, hh)

            def proj_chunk1(dst_kind, w_sb, hh, o2):
                # pair-1 projection thunk (one 512-col o2 half, ~2.1us of PE):
                # allocates and fully consumes its PSUM tile within one thunk
                # so the flow ring can rotate freely between thunk drains
                ps = flow.tile([128, 512], F32, tag="flow", name="flow")
                for kc in range(KC):
                    nc.tensor.matmul(
                        ps[:],
                        w_sb[:, kc, :],
                        x_sb[kc][
                            :,
                            hh * 1024 + o2 * 512 : hh * 1024 + (o2 + 1) * 512,
                        ],
                        start=(kc == 0),
                        stop=(kc == KC - 1),
                    )
                cols = slice(hh * 1024 + o2 * 512, hh * 1024 + (o2 + 1) * 512)
                if dst_kind == "q":
                    nc.vector.tensor_copy(q_sb[1][:, cols], ps[:])
                elif dst_kind == "k":
                    nc.vector.tensor_copy(kpad[1][0][0:64, cols], ps[0:64, :])
                    nc.vector.tensor_copy(
                        kpad[1][1][64:128, cols], ps[64:128, :]
                    )
                else:
                    nc.vector.tensor_copy(vt_tmps[1][:, cols], ps[:])

            def v_tr(p, hd):
                # one big xbar transpose per head: [64, 2048] -> blocked
                # [128, 16, 64] (partition-wrapped transpose); head1 lands
                # at cols 192:256 (after the ones block)
                dst_c = 0 if hd == 0 else 192
                nc.sync.dma_start_transpose(
                    v_sb[p][:, :, dst_c : dst_c + 64],
                    vt_tmps[p][hd * 64 : (hd + 1) * 64, :],
                )

            # pair 0 inline; pair 1 deferred into pair-0's attention loop
            proj_pair0()
            v_tr(0, 0)
            v_tr(0, 1)

            def mk(f, *a):
                return lambda: f(*a)

            for dst_kind, w_sb in (
                ("q", wq_sb[1]),
                ("k", wk_sb[1]),
                ("v", wv_sb[1]),
            ):
                for hh in range(2):
                    for o2 in range(2):
                        backlog.append(mk(proj_chunk1, dst_kind, w_sb, hh, o2))
            backlog.append(mk(v_tr, 1, 0))
            backlog.append(mk(v_tr, 1, 1))

            # --- normalization (channel-major, no transposes) --------------
            def norm(p, Q, y_aug, c0, c1, sq=0):
                """Normalize y_aug columns [c0:c1) into yT_sb[p].
                y_aug[0] rows 0:64 = y0, rows 64:128 = d0 (replicated);
                y_aug[1] rows 0:64 = d1, rows 64:128 = y1."""
                qlo = Q * 512
                r = normp.tile([128, 512], F32, tag="r", name="r")
                rs = normp.tile([128, 512], F32, tag="rs", name="rs")
                # ACT-table reciprocal via exp(-ln d): the DVE reciprocal is
                # ~4.3 cyc/el (1.6us per block) and gates the quarter
                # pipeline; ACT sits next to PSUM at ~1 el/cyc.  d >= 1
                # always (diagonal exp(0)=1 is in every row sum), so ln is
                # safe and exp(-ln d) is in (0, 1].
                ln0 = normp.tile([128, 512], F32, tag="lnd", name="lnd")
                nc.scalar.activation(
                    ln0[64:128, c0:c1],
                    y_aug[0][64:128, c0:c1],
                    mybir.ActivationFunctionType.Ln,
                )
                nc.scalar.activation(
                    ln0[0:64, c0:c1],
                    y_aug[1][0:64, c0:c1],
                    mybir.ActivationFunctionType.Ln,
                )
                nc.scalar.activation(
                    r[64:128, c0:c1],
                    ln0[64:128, c0:c1],
                    mybir.ActivationFunctionType.Exp,
                    scale=-1.0,
                )
                nc.scalar.activation(
                    r[0:64, c0:c1],
                    ln0[0:64, c0:c1],
                    mybir.ActivationFunctionType.Exp,
                    scale=-1.0,
                )
                nc.sync.dma_start(rs[0:64, c0:c1], r[64:128, c0:c1])
                nc.sync.dma_start(rs[64:128, c0:c1], r[0:64, c0:c1])
                nc.vector.tensor_mul(
                    yT_sb[p][0:64, qlo + c0 : qlo + c1],
                    y_aug[0][0:64, c0:c1],
                    rs[0:64, c0:c1],
                )
                nc.vector.tensor_mul(
                    yT_sb[p][64:128, qlo + c0 : qlo + c1],
                    y_aug[1][64:128, c0:c1],
                    rs[64:128, c0:c1],
                )

            def outproj(tb, act=False):
                def f():
                    o_sb = outp.tile([128, C], F16, tag="osb", name="osb")
                    for nch in range(2):
                        ps = flow.tile([128, 512], F32, tag="flow", name="flow")
                        for p in range(NPAIRS):
                            nc.tensor.matmul(
                                ps[:],
                                yT_sb[p][:, tb * 128 : (tb + 1) * 128],
                                wo_sb[p][:, nch * 512 : (nch + 1) * 512],
                                start=(p == 0),
                                stop=(p == NPAIRS - 1),
                            )
                        dst = o_sb[:, nch * 512 : (nch + 1) * 512]
                        if act:
                            # tail blocks: ACT is idle after the last exp
                            nc.scalar.copy(dst, ps[:])
                        else:
                            # attention phase: ACT is exp-saturated
                            nc.vector.tensor_copy(dst, ps[:])
                    nc.gpsimd.dma_start(out[tb], o_sb[:])

                return f

            # --- attention -------------------------------------------------
            it_count = [0]
            for p in range(NPAIRS):
                for Q in range(4):  # q quarters of 512
                    qlo = Q * 512
                    final = (p, Q) == (1, 3)
                    y_aug = [
                        acc.tile([128, 512], F32, tag="acc", name="acc")
                        for _ in range(2)
                    ]
                    nkb = 4 * Q + 4
                    # software pipeline: AV lags scores by TWO iterations so
                    # the PE never waits on the ACT exp of the tile it is
                    # about to consume.  Each PSUM tile holds BOTH heads'
                    # score chunk (cols 0:512 / 512:1024) so a single
                    # (strided) ACT exp covers them.
                    groups = []  # per-iteration AV work

                    def flush_one(p=p, qlo=qlo, nkb=nkb, y_aug=y_aug,
                                  groups=groups):
                        for (kb, hd, at, q0, wdt) in groups.pop(0):
                            nc.tensor.matmul(
                                y_aug[hd][:, q0 - qlo : q0 - qlo + wdt],
                                v_sb[p][:, kb, hd * 128 : (hd + 1) * 128],
                                at[:, hd * 512 : hd * 512 + wdt],
                                start=(kb == 0),
                                stop=(kb == nkb - 1),
                            )

                    for kb in range(nkb):
                        qs = max(kb * 128, qlo)  # global q start
                        wdt = qlo + 512 - qs
                        ps = flow.tile([128, 1024], F32, tag="flow", name="flow")
                        for hd in range(2):
                            nc.tensor.matmul(
                                ps[:, hd * 512 : hd * 512 + wdt],
                                kpad[p][hd][:, kb * 128 : (kb + 1) * 128],
                                q_sb[p][:, qs : qs + wdt],
                                start=True,
                                stop=True,
                            )
                        if len(groups) >= 2:
                            flush_one()
                        if final and kb >= 14:
                            # block kb-14's accumulation completed in that
                            # flush (its diagonal k-block is kb-2)
                            norm(1, 3, y_aug, (kb - 14) * 128,
                                 (kb - 13) * 128, sq=kb % 2)
                            if kb == 15:
                                outproj(12)()
                        if not final and kb == nkb - 1:
                            # cols 0:256 of this quarter completed in the
                            # flush above (their diagonal blocks are <= kb-2)
                            norm(p, Q, y_aug, 0, 256, sq=0)
                        # paced drain (every other iteration): the thunks are
                        # PE filler for the ACT-bound exp pipeline; draining
                        # faster front-loads them and leaves the late
                        # iterations exposed to exp latency
                        if it_count[0] % 2 == 0:
                            drain_one()
                        it_count[0] += 1
                        at = attnp.tile([128, 1024], BF16, tag="attn",
                                        name="attn")
                        if wdt == 512:
                            nc.scalar.activation(
                                at[:], ps[:], mybir.ActivationFunctionType.Exp
                            )
                        else:
                            v2 = ps[:].rearrange("p (h w) -> p h w", h=2)
                            a2 = at[:].rearrange("p (h w) -> p h w", h=2)
                            nc.scalar.activation(
                                a2[:, :, 0:wdt],
                                v2[:, :, 0:wdt],
                                mybir.ActivationFunctionType.Exp,
                            )
                        if kb * 128 >= qlo:
                            # diagonal block: causal mask, both heads
                            for hd in range(2):
                                nc.vector.tensor_mul(
                                    at[:, hd * 512 : hd * 512 + 128],
                                    at[:, hd * 512 : hd * 512 + 128],
                                    tri[:],
                                )
                        groups.append(
                            [(kb, hd, at, qs, wdt) for hd in range(2)]
                        )
                    # drain the 2 lagged iterations
                    if final:
                        flush_one()  # kb=14
                        norm(1, 3, y_aug, 256, 384, sq=0)
                        outproj(13, act=True)()
                        flush_one()  # kb=15
                        norm(1, 3, y_aug, 384, 512, sq=1)
                        outproj(14, act=True)()
                        outproj(15, act=True)()
                    else:
                        flush_one()  # group nkb-2
                        norm(p, Q, y_aug, 256, 384, sq=1)
                        flush_one()  # group nkb-1
                        norm(p, Q, y_aug, 384, 512, sq=0)
                        if p == 1:
                            # both pairs' yT ready for this quarter's blocks
                            for tb in range(4 * Q, 4 * Q + 4):
                                backlog.append(outproj(tb))
            drain_all()

    split_waits(nc)
    nc.finalize()
    return nc


def _get_nc():
    global _nc_cache
    if _nc_cache is None:
        _nc_cache = build_nc()
    return _nc_cache


def _prep_core_inputs(x, Wq, Wk, Wv, Wo, core):
    b, g = core // 4, core % 4
    bf = ml_dtypes.bfloat16
    xT_host = np.ascontiguousarray(x[b].T).reshape(KC, 128, T).astype(bf)

    def pack_w(Wm, scale=1.0):
        # [NPAIRS, 128, KC, 128]: lhsT chunks; columns = 2 heads' channels
        outw = np.empty((NPAIRS, 128, KC, 128), dtype=bf)
        for p in range(NPAIRS):
            h0 = 4 * g + 2 * p
            blk = Wm[h0 * D : (h0 + 2) * D, :].T * scale  # [C, 128]
            outw[p] = blk.reshape(KC, 128, 128).transpose(1, 0, 2).astype(bf)
        return outw

    scale = 1.0 / np.sqrt(D)
    wo_host = np.empty((NPAIRS, 128, C), dtype=bf)
    for p in range(NPAIRS):
        h0 = 4 * g + 2 * p
        wo_host[p] = Wo[:, h0 * D : (h0 + 2) * D].T.astype(bf)
    return {
        "xT": xT_host,
        "wq": pack_w(Wq, scale),
        "wk": pack_w(Wk),
        "wv": pack_w(Wv),
        "wo": wo_host,
    }


def _ensure_ntff_hook():
    """antenv.axon_hooks is missing from this image; shim it and register
    the ctypes NTFF profile hook so trace=True captures profiles."""
    import types

    if "antenv.axon_hooks" in sys.modules:
        return
    mod = types.ModuleType("antenv.axon_hooks")
    mod._hook = None

    def set_axon_ntff_profile_hook(h):
        mod._hook = h

    def get_axon_ntff_profile_hook():
        return mod._hook

    mod.set_axon_ntff_profile_hook = set_axon_ntff_profile_hook
    mod.get_axon_ntff_profile_hook = get_axon_ntff_profile_hook
    sys.modules["antenv.axon_hooks"] = mod
    try:
        from trn_agent_boot.trn_boot import _ntff_profile_via_ctypes

        mod._hook = _ntff_profile_via_ctypes("/opt/axon/libaxon_pjrt.so")
    except Exception as e:
        print(f"ntff hook setup failed: {e}")


def kernel(x, Wq, Wk, Wv, Wo, _trace=False, _tracedir=None):
    if _trace:
        _ensure_ntff_hook()
    x = np.asarray(x, dtype=np.float32)
    Wq, Wk, Wv, Wo = (np.asarray(w, dtype=np.float32) for w in (Wq, Wk, Wv, Wo))
    nc = _get_nc()
    in_maps = [_prep_core_inputs(x, Wq, Wk, Wv, Wo, c) for c in range(NCORES)]
    res = run_bass_kernel_spmd(
        nc, in_maps, core_ids=list(range(NCORES)), trace=_trace, tmpdir=_tracedir
    )
    out = np.zeros((NB, T, C), dtype=np.float32)
    for c in range(NCORES):
        out[c // 4] += res.results[c]["out"].astype(np.float32).reshape(T, C)
    if _trace:
        kernel._last_results = res
    return out


# revision 27
# speedup vs baseline: 1.1066x; 1.0009x over previous
"""Causal self-attention (B=2, T=2048, C=1024, H=16, D=64) on 8 trn2 cores.

Sharding: data-parallel over B (2) x tensor-parallel over head groups (4).
Core c handles batch b = c // 4 and heads [4g, 4g+4) with g = c % 4.
Each core computes a partial output  y_local @ Wo_local.T  of shape [T, C];
the host sums the 4 partials per batch.

Per-core kernel (bf16 matmuls, f32 PSUM):
  kpadT  [128, T] per (pair, head): head's 64 k-channels + 64 zero rows,
         written directly by the k-projection copies (no separate pad pass)
  scoresT[k, q] = kpad.T @ qT     (q needs NO padding: kpad's zero rows
                                   nullify the other head's q rows)
  attnT = exp(scoresT)            (no max subtraction: |scores| <~ 16 here)
  AV with v blocks [v0 | ones64] and [ones64 | v1]: the 64 ones columns
  replicate the softmax denominator across 64 partitions FOR FREE (matmul
  cost is N-bound; M-padding costs nothing).  Normalize entirely in
  channel-major layout: DVE reciprocal of the denominator block, one tiny
  SBUF->SBUF DMA partition shift, DVE multiply into yT.  No PE transposes,
  no DMA xbar transposes, no token-major round trip.

Perf notes:
  - matmul streaming measured at ~1.96 cols/ns (power-throttled 2.4 GHz);
    cost is N-bound, so K=128 padding is free.
  - software pipeline with AV lagging scores by TWO iterations so the PE
    never waits on the ~1.3us ACT exp of the tile it consumes.
  - projections run per-chunk interleaved (q,k,v share each arriving x
    chunk; 3 concurrent PSUM accumulators) so the start is DMA-paced, and
    the very first matmul needs only wq chunk0 + a 128KB x piece.
  - pair-1 projections/outprojs are deferred thunks drained one per
    attention iteration so dense PE work fills any exp-latency gaps.
  - outproj PSUM->SBUF copies run on DVE during attention (ACT is
    exp-saturated); the final blocks use ACT (idle at the tail).
  - output written f16 (host sums partials in f32): halves HBM writes.
  - final quarter (1,3) normalizes + out-projects per 128-col block as
    each block's accumulation completes, so the serial tail is ~3us.
"""

import sys

sys.path.insert(0, "/opt/trn_rl_repo")

import numpy as np
import ml_dtypes

import concourse.bass as bass
import concourse.tile as tile
from concourse import mybir
from concourse.bass_utils import run_bass_kernel_spmd
from concourse.masks import make_upper_triangular

BF16 = mybir.dt.bfloat16
F16 = mybir.dt.float16
F32 = mybir.dt.float32

T = 2048
C = 1024
H = 16
D = 64
NB = 2  # batch
NCORES = 8
NPAIRS = 2  # head pairs per core
KC = C // 128  # 8 contraction chunks for projections
NTB = T // 128  # 16 token blocks

_nc_cache = None


def split_waits(nc, max_waits=1):
    """This walrus build rejects instructions with more than one semaphore
    wait; move excess waits onto same-engine NOPs inserted just before."""
    for fn in nc.m.functions:
        for bb in fn.blocks:
            insts = bb.instructions
            new_list = []
            changed = False
            for inst in insts:
                si = inst.sync_info
                if si is not None and len(si.on_wait) > max_waits:
                    waits = list(si.on_wait)
                    extra, keep = waits[:-max_waits], waits[-max_waits:]
                    k = 0
                    while extra:
                        chunk, extra = extra[:max_waits], extra[max_waits:]
                        nop = mybir.InstNoOp(
                            name=f"{inst.name}-wsplit{k}", engine=inst.engine
                        )
                        nop.sync_info = mybir.SyncInfo(on_wait=chunk, on_update=[])
                        new_list.append(nop)
                        changed = True
                        k += 1
                    inst.sync_info = mybir.SyncInfo(
                        on_wait=keep, on_update=list(si.on_update)
                    )
                new_list.append(inst)
            if changed:
                bb.instructions = new_list


def build_nc():
    nc = bass.Bass()

    xT = nc.dram_tensor("xT", [KC, 128, T], BF16, kind="ExternalInput")
    wq = nc.dram_tensor("wq", [NPAIRS, 128, KC, 128], BF16, kind="ExternalInput")
    wk = nc.dram_tensor("wk", [NPAIRS, 128, KC, 128], BF16, kind="ExternalInput")
    wv = nc.dram_tensor("wv", [NPAIRS, 128, KC, 128], BF16, kind="ExternalInput")
    wo = nc.dram_tensor("wo", [NPAIRS, 128, C], BF16, kind="ExternalInput")
    # f16 output: halves HBM-write traffic; host sums partials in f32
    out = nc.dram_tensor("out", [NTB, 128, C], F16, kind="ExternalOutput")

    with tile.TileContext(nc) as tc:
        with (
            tc.tile_pool(name="const", bufs=1) as const,
            tc.tile_pool(name="persist", bufs=1) as persist,
            tc.tile_pool(name="temps", bufs=1) as temps,
            tc.tile_pool(name="attnp", bufs=6) as attnp,
            tc.tile_pool(name="normp", bufs=4) as normp,
            tc.tile_pool(name="outp", bufs=3) as outp,
            tc.tile_pool(name="flow", bufs=2, space="PSUM") as flow,
            tc.tile_pool(name="acc", bufs=4, space="PSUM") as acc,
        ):
            # Deferred-emission backlog: thunks emitted one per matmul-loop
            # iteration so latency-bound chains overlap dense matmul work.
            backlog = []

            def drain_one():
                if backlog:
                    backlog.pop(0)()

            def drain_all():
                while backlog:
                    backlog.pop(0)()

            # --- SBUF tensors ----------------------------------------------
            wq_sb, wk_sb, wv_sb, wo_sb = [], [], [], []
            for p in range(NPAIRS):
                for lst, nm in ((wq_sb, "wq"), (wk_sb, "wk"), (wv_sb, "wv")):
                    lst.append(
                        persist.tile(
                            [128, KC, 128], BF16, tag=f"{nm}{p}", name=f"{nm}{p}"
                        )
                    )
                wo_sb.append(
                    persist.tile([128, C], BF16, tag=f"wo{p}", name=f"wo{p}")
                )
            x_sb = [
                persist.tile([128, T], BF16, tag=f"x{kc}", name=f"x{kc}")
                for kc in range(KC)
            ]
            # qT unpadded (both heads stacked); kpad per head (zero rows kill
            # the other head's q rows in the scores contraction)
            q_sb, kpad, v_sb, yT_sb = [], [], [], []
            for p in range(NPAIRS):
                q_sb.append(persist.tile([128, T], BF16, tag=f"qT{p}", name=f"qT{p}"))
                kpad.append(
                    [
                        persist.tile([128, T], BF16, tag=f"kp{p}{hd}", name=f"kp{p}{hd}")
                        for hd in range(2)
                    ]
                )
                v_sb.append(
                    persist.tile([128, NTB, 256], BF16, tag=f"v{p}", name=f"v{p}")
                )
                yT_sb.append(persist.tile([128, T], BF16, tag=f"yT{p}", name=f"yT{p}"))
            vt_tmps = [
                temps.tile([128, T], BF16, tag=f"vt{p}", name=f"vt{p}")
                for p in range(NPAIRS)
            ]

            # --- input DMAs ------------------------------------------------
            # issue order ~= arrival order (the DMA engines serialize
            # globally): interleave weight chunks with x chunks to match the
            # per-chunk projection consumption; first x piece is 128KB so
            # the first matmul starts as early as possible.
            loads = [
                (wq_sb[0][:], wq[0]),
                (x_sb[0][:, 0:512], xT[0][:, 0:512]),
                (wk_sb[0][:], wk[0]),
                (x_sb[1][:, 0:512], xT[1][:, 0:512]),
                (wv_sb[0][:], wv[0]),
            ]
            # x in 512-col pieces, in exact projection consumption order
            for kc in range(2, KC):
                loads.append((x_sb[kc][:, 0:512], xT[kc][:, 0:512]))
            for c0 in (512, 1024, 1536):
                for kc in range(KC):
                    loads.append(
                        (x_sb[kc][:, c0 : c0 + 512], xT[kc][:, c0 : c0 + 512])
                    )
            loads += [
                (wo_sb[0][:], wo[0]),
                (wq_sb[1][:], wq[1]),
                (wk_sb[1][:], wk[1]),
                (wv_sb[1][:], wv[1]),
                (wo_sb[1][:], wo[1]),
            ]
            qrr = [nc.sync, nc.scalar, nc.gpsimd]
            for i, (dst, src_ap) in enumerate(loads):
                qrr[i % 3].dma_start(dst, src_ap)

            # --- constants / one-time zero fills ---------------------------
            # multiplicative causal mask for diagonal blocks of attnT:
            # valid where k_local <= q_local (upper triangle incl diag)
            tri = const.tile([128, 128], BF16, tag="tri", name="tri")
            make_upper_triangular(nc, tri[:], val=1.0, diag=True)
            # kpad zero halves (data halves written by the k-proj copies)
            for p in range(NPAIRS):
                nc.gpsimd.memset(kpad[p][0][64:128, :], 0.0)
                nc.gpsimd.memset(kpad[p][1][0:64, :], 0.0)
            # v blocks: [v0 | ones] and [ones | v1] per 128-col head block;
            # the ones columns are contiguous in the middle
            for p in range(NPAIRS):
                nc.gpsimd.memset(v_sb[p][:, :, 64:192], 1.0)

            # --- projections -----------------------------------------------
            def proj_copy(p, dst_kind, ps, hh):
                cols = slice(hh * 1024, (hh + 1) * 1024)
                if dst_kind == "q":
                    nc.vector.tensor_copy(q_sb[p][:, cols], ps[:])
                elif dst_kind == "k":
                    nc.vector.tensor_copy(kpad[p][0][0:64, cols], ps[0:64, :])
                    nc.vector.tensor_copy(
                        kpad[p][1][64:128, cols], ps[64:128, :]
                    )
                else:
                    if p == 0:
                        nc.scalar.copy(vt_tmps[p][:, cols], ps[:])
                    else:
                        nc.vector.tensor_copy(vt_tmps[p][:, cols], ps[:])

            def proj_unit0(hh, o2):
                # pair-0 runs inline at startup, per-chunk interleaved over
                # q/k/v (3 concurrent [128,512] accumulators on the acc ring)
                # so consumption is paced to DMA arrival and the first matmul
                # needs only wq0 + a 128KB x piece.
                c0 = hh * 1024 + o2 * 512
                psq = acc.tile([128, 512], F32, tag="acc", name="pq")
                psk = acc.tile([128, 512], F32, tag="acc", name="pk")
                psv = acc.tile([128, 512], F32, tag="acc", name="pv")
                for kc in range(KC):
                    for ps, w_sb in (
                        (psq, wq_sb[0]),
                        (psk, wk_sb[0]),
                        (psv, wv_sb[0]),
                    ):
                        nc.tensor.matmul(
                            ps[:],
                            w_sb[:, kc, :],
                            x_sb[kc][:, c0 : c0 + 512],
                            start=(kc == 0),
                            stop=(kc == KC - 1),
                        )
                sl = slice(c0, c0 + 512)
                nc.vector.tensor_copy(q_sb[0][:, sl], psq[:])
                nc.vector.tensor_copy(kpad[0][0][0:64, sl], psk[0:64, :])
                nc.vector.tensor_copy(kpad[0][1][64:128, sl], psk[64:128, :])
                nc.scalar.copy(vt_tmps[0][:, sl], psv[:])

            def proj_pair0():
                for hh in range(2):
                    for o2 in range(2):
                        proj_unit0(hh, o2)

            def proj_chunk1(dst_kind, w_sb, hh, o2):
                # pair-1 projection thunk (one 512-col o2 half, ~2.1us of PE):
                # allocates and fully consumes its PSUM tile within one thunk
                # so the flow ring can rotate freely between thunk drains
                ps = flow.tile([128, 512], F32, tag="flow", name="flow")
                for kc in range(KC):
                    nc.tensor.matmul(
                        ps[:],
                        w_sb[:, kc, :],
                        x_sb[kc][
                            :,
                            hh * 1024 + o2 * 512 : hh * 1024 + (o2 + 1) * 512,
                        ],
                        start=(kc == 0),
                        stop=(kc == KC - 1),
                    )
                cols = slice(hh * 1024 + o2 * 512, hh * 1024 + (o2 + 1) * 512)
                if dst_kind == "q":
                    nc.vector.tensor_copy(q_sb[1][:, cols], ps[:])
                elif dst_kind == "k":
                    nc.vector.tensor_copy(kpad[1][0][0:64, cols], ps[0:64, :])
                    nc.vector.tensor_copy(
                        kpad[1][1][64:128, cols], ps[64:128, :]
                    )
                else:
                    nc.vector.tensor_copy(vt_tmps[1][:, cols], ps[:])

            def v_tr(p, hd):
                # one big xbar transpose per head: [64, 2048] -> blocked
                # [128, 16, 64] (partition-wrapped transpose); head1 lands
                # at cols 192:256 (after the ones block)
                dst_c = 0 if hd == 0 else 192
                nc.sync.dma_start_transpose(
                    v_sb[p][:, :, dst_c : dst_c + 64],
                    vt_tmps[p][hd * 64 : (hd + 1) * 64, :],
                )

            # pair 0 inline; pair 1 deferred into pair-0's attention loop
            proj_pair0()
            v_tr(0, 0)
            v_tr(0, 1)

            def mk(f, *a):
                return lambda: f(*a)

            for dst_kind, w_sb in (
                ("q", wq_sb[1]),
                ("k", wk_sb[1]),
                ("v", wv_sb[1]),
            ):
                for hh in range(2):
                    for o2 in range(2):
                        backlog.append(mk(proj_chunk1, dst_kind, w_sb, hh, o2))
            backlog.append(mk(v_tr, 1, 0))
            backlog.append(mk(v_tr, 1, 1))

            # --- normalization (channel-major, no transposes) --------------
            def norm(p, Q, y_aug, c0, c1, sq=0):
                """Normalize y_aug columns [c0:c1) into yT_sb[p].
                y_aug[0] rows 0:64 = y0, rows 64:128 = d0 (replicated);
                y_aug[1] rows 0:64 = d1, rows 64:128 = y1."""
                qlo = Q * 512
                r = normp.tile([128, 512], F32, tag="r", name="r")
                rs = normp.tile([128, 512], F32, tag="rs", name="rs")
                # ACT-table reciprocal via exp(-ln d): the DVE reciprocal is
                # ~4.3 cyc/el (1.6us per block) and gates the quarter
                # pipeline; ACT sits next to PSUM at ~1 el/cyc.  d >= 1
                # always (diagonal exp(0)=1 is in every row sum), so ln is
                # safe and exp(-ln d) is in (0, 1].
                ln0 = normp.tile([128, 512], F32, tag="lnd", name="lnd")
                nc.scalar.activation(
                    ln0[64:128, c0:c1],
                    y_aug[0][64:128, c0:c1],
                    mybir.ActivationFunctionType.Ln,
                )
                nc.scalar.activation(
                    ln0[0:64, c0:c1],
                    y_aug[1][0:64, c0:c1],
                    mybir.ActivationFunctionType.Ln,
                )
                nc.scalar.activation(
                    r[64:128, c0:c1],
                    ln0[64:128, c0:c1],
                    mybir.ActivationFunctionType.Exp,
                    scale=-1.0,
                )
                nc.scalar.activation(
                    r[0:64, c0:c1],
                    ln0[0:64, c0:c1],
                    mybir.ActivationFunctionType.Exp,
                    scale=-1.0,
                )
                nc.sync.dma_start(rs[0:64, c0:c1], r[64:128, c0:c1])
                nc.sync.dma_start(rs[64:128, c0:c1], r[0:64, c0:c1])
                nc.vector.tensor_mul(
                    yT_sb[p][0:64, qlo + c0 : qlo + c1],
                    y_aug[0][0:64, c0:c1],
                    rs[0:64, c0:c1],
                )
                nc.vector.tensor_mul(
                    yT_sb[p][64:128, qlo + c0 : qlo + c1],
                    y_aug[1][64:128, c0:c1],
                    rs[64:128, c0:c1],
                )

            def outproj(tb, act=False):
                def f():
                    o_sb = outp.tile([128, C], F16, tag="osb", name="osb")
                    for nch in range(2):
                        ps = flow.tile([128, 512], F32, tag="flow", name="flow")
                        for p in range(NPAIRS):
                            nc.tensor.matmul(
                                ps[:],
                                yT_sb[p][:, tb * 128 : (tb + 1) * 128],
                                wo_sb[p][:, nch * 512 : (nch + 1) * 512],
                                start=(p == 0),
                                stop=(p == NPAIRS - 1),
                            )
                        dst = o_sb[:, nch * 512 : (nch + 1) * 512]
                        if act:
                            # tail blocks: ACT is idle after the last exp
                            nc.scalar.copy(dst, ps[:])
                        else:
                            # attention phase: ACT is exp-saturated
                            nc.vector.tensor_copy(dst, ps[:])
                    nc.gpsimd.dma_start(out[tb], o_sb[:])

                return f

            # --- attention -------------------------------------------------
            # normlog: priority queue for deferred norm pieces, drained one
            # per iteration AFTER the exp emission so the norm's ACT ops
            # never delay the exp that the lag-2 AV pipeline waits on.  The
            # acc ring (4 bufs) gives each quarter's y_aug a full quarter of
            # slack before reuse, so norms can lag by a few iterations.
            normlog = []

            def drain_norm():
                if normlog:
                    normlog.pop(0)()

            it_count = [0]
            for p in range(NPAIRS):
                for Q in range(4):  # q quarters of 512
                    qlo = Q * 512
                    final = (p, Q) == (1, 3)
                    y_aug = [
                        acc.tile([128, 512], F32, tag="acc", name="acc")
                        for _ in range(2)
                    ]
                    nkb = 4 * Q + 4
                    # software pipeline: AV lags scores by TWO iterations so
                    # the PE never waits on the ACT exp of the tile it is
                    # about to consume.  Each PSUM tile holds BOTH heads'
                    # score chunk (cols 0:512 / 512:1024) so a single
                    # (strided) ACT exp covers them.
                    groups = []  # per-iteration AV work

                    def flush_one(p=p, qlo=qlo, nkb=nkb, y_aug=y_aug,
                                  groups=groups):
                        for (kb, hd, at, q0, wdt) in groups.pop(0):
                            nc.tensor.matmul(
                                y_aug[hd][:, q0 - qlo : q0 - qlo + wdt],
                                v_sb[p][:, kb, hd * 128 : (hd + 1) * 128],
                                at[:, hd * 512 : hd * 512 + wdt],
                                start=(kb == 0),
                                stop=(kb == nkb - 1),
                            )

                    for kb in range(nkb):
                        qs = max(kb * 128, qlo)  # global q start
                        wdt = qlo + 512 - qs
                        ps = flow.tile([128, 1024], F32, tag="flow", name="flow")
                        for hd in range(2):
                            nc.tensor.matmul(
                                ps[:, hd * 512 : hd * 512 + wdt],
                                kpad[p][hd][:, kb * 128 : (kb + 1) * 128],
                                q_sb[p][:, qs : qs + wdt],
                                start=True,
                                stop=True,
                            )
                        if len(groups) >= 2:
                            flush_one()
                        if not final and kb == nkb - 1:
                            # cols 0:256 of this quarter completed in the
                            # flush above (their diagonal blocks are <= kb-2)
                            normlog.append(mk(norm, p, Q, y_aug, 0, 256, 0))
                        at = attnp.tile([128, 1024], BF16, tag="attn",
                                        name="attn")
                        if wdt == 512:
                            nc.scalar.activation(
                                at[:], ps[:], mybir.ActivationFunctionType.Exp
                            )
                        else:
                            v2 = ps[:].rearrange("p (h w) -> p h w", h=2)
                            a2 = at[:].rearrange("p (h w) -> p h w", h=2)
                            nc.scalar.activation(
                                a2[:, :, 0:wdt],
                                v2[:, :, 0:wdt],
                                mybir.ActivationFunctionType.Exp,
                            )
                        if kb * 128 >= qlo:
                            # diagonal block: causal mask, both heads
                            for hd in range(2):
                                nc.vector.tensor_mul(
                                    at[:, hd * 512 : hd * 512 + 128],
                                    at[:, hd * 512 : hd * 512 + 128],
                                    tri[:],
                                )
                        # drains AFTER the exp emission: deferred norm ACT
                        # ops and PE-filler thunks must not delay the exp
                        # that the lag-2 AV pipeline is waiting on
                        drain_norm()
                        if it_count[0] % 2 == 0:
                            drain_one()
                        it_count[0] += 1
                        if final and kb >= 14:
                            # block kb-14's accumulation completed in this
                            # iteration's flush (its diagonal k-block, kb-2)
                            norm(1, 3, y_aug, (kb - 14) * 128,
                                 (kb - 13) * 128, sq=kb % 2)
                            if kb == 15:
                                outproj(12)()
                        groups.append(
                            [(kb, hd, at, qs, wdt) for hd in range(2)]
                        )
                    # drain the 2 lagged iterations
                    if final:
                        flush_one()  # kb=14
                        norm(1, 3, y_aug, 256, 384, sq=0)
                        outproj(13, act=True)()
                        flush_one()  # kb=15
                        norm(1, 3, y_aug, 384, 512, sq=1)
                        outproj(14, act=True)()
                        outproj(15, act=True)()
                    else:
                        flush_one()  # group nkb-2
                        normlog.append(mk(norm, p, Q, y_aug, 256, 384, 1))
                        flush_one()  # group nkb-1
                        normlog.append(mk(norm, p, Q, y_aug, 384, 512, 0))
                        if p == 1:
                            # both pairs' yT ready for this quarter's blocks
                            for tb in range(4 * Q, 4 * Q + 4):
                                backlog.append(outproj(tb))
            while normlog:
                normlog.pop(0)()
            drain_all()

    split_waits(nc)
    nc.finalize()
    return nc


def _get_nc():
    global _nc_cache
    if _nc_cache is None:
        _nc_cache = build_nc()
    return _nc_cache


def _prep_core_inputs(x, Wq, Wk, Wv, Wo, core):
    b, g = core // 4, core % 4
    bf = ml_dtypes.bfloat16
    xT_host = np.ascontiguousarray(x[b].T).reshape(KC, 128, T).astype(bf)

    def pack_w(Wm, scale=1.0):
        # [NPAIRS, 128, KC, 128]: lhsT chunks; columns = 2 heads' channels
        outw = np.empty((NPAIRS, 128, KC, 128), dtype=bf)
        for p in range(NPAIRS):
            h0 = 4 * g + 2 * p
            blk = Wm[h0 * D : (h0 + 2) * D, :].T * scale  # [C, 128]
            outw[p] = blk.reshape(KC, 128, 128).transpose(1, 0, 2).astype(bf)
        return outw

    scale = 1.0 / np.sqrt(D)
    wo_host = np.empty((NPAIRS, 128, C), dtype=bf)
    for p in range(NPAIRS):
        h0 = 4 * g + 2 * p
        wo_host[p] = Wo[:, h0 * D : (h0 + 2) * D].T.astype(bf)
    return {
        "xT": xT_host,
        "wq": pack_w(Wq, scale),
        "wk": pack_w(Wk),
        "wv": pack_w(Wv),
        "wo": wo_host,
    }


def _ensure_ntff_hook():
    """antenv.axon_hooks is missing from this image; shim it and register
    the ctypes NTFF profile hook so trace=True captures profiles."""
    import types

    if "antenv.axon_hooks" in sys.modules:
        return
    mod = types.ModuleType("antenv.axon_hooks")
    mod._hook = None

    def set_axon_ntff_profile_hook(h):
        mod._hook = h

    def get_axon_ntff_profile_hook():
        return mod._hook

    mod.set_axon_ntff_profile_hook = set_axon_ntff_profile_hook
    mod.get_axon_ntff_profile_hook = get_axon_ntff_profile_hook
    sys.modules["antenv.axon_hooks"] = mod
    try:
        from trn_agent_boot.trn_boot import _ntff_profile_via_ctypes

        mod._hook = _ntff_profile_via_ctypes("/opt/axon/libaxon_pjrt.so")
    except Exception as e:
        print(f"ntff hook setup failed: {e}")


def kernel(x, Wq, Wk, Wv, Wo, _trace=False, _tracedir=None):
    if _trace:
        _ensure_ntff_hook()
    x = np.asarray(x, dtype=np.float32)
    Wq, Wk, Wv, Wo = (np.asarray(w, dtype=np.float32) for w in (Wq, Wk, Wv, Wo))
    nc = _get_nc()
    in_maps = [_prep_core_inputs(x, Wq, Wk, Wv, Wo, c) for c in range(NCORES)]
    res = run_bass_kernel_spmd(
        nc, in_maps, core_ids=list(range(NCORES)), trace=_trace, tmpdir=_tracedir
    )
    out = np.zeros((NB, T, C), dtype=np.float32)
    for c in range(NCORES):
        out[c // 4] += res.results[c]["out"].astype(np.float32).reshape(T, C)
    if _trace:
        kernel._last_results = res
    return out


# revision 28
# speedup vs baseline: 1.3795x; 1.2466x over previous
"""Causal self-attention (B=2, T=2048, C=1024, H=16, D=64) on 8 trn2 cores.

BASELINE RECONSTRUCTION (172968 ns) — restore to kernel.py if needed.

Sharding: data-parallel over B (2) x tensor-parallel over head groups (4).
Core c handles batch b = c // 4 and heads [4g, 4g+4) with g = c % 4.
Each core computes a partial output  y_local @ Wo_local.T  of shape [T, C];
the host sums the 4 partials per batch.
"""

import sys

sys.path.insert(0, "/opt/trn_rl_repo")

import numpy as np
import ml_dtypes

import concourse.bass as bass
import concourse.tile as tile
from concourse import mybir
from concourse.bass_utils import run_bass_kernel_spmd
from concourse.masks import make_identity, make_upper_triangular

BF16 = mybir.dt.bfloat16
F16 = mybir.dt.float16
F32 = mybir.dt.float32

T = 2048
C = 1024
H = 16
D = 64
NB = 2  # batch
NCORES = 8
NPAIRS = 2  # head pairs per core
KC = C // 128  # 8 contraction chunks for projections
NTB = T // 128  # 16 token blocks
HALF = T // 2  # 1024

_nc_cache = None


def split_waits(nc, max_waits=1):
    """This walrus build rejects instructions with more than one semaphore
    wait; move excess waits onto same-engine NOPs inserted just before."""
    for fn in nc.m.functions:
        for bb in fn.blocks:
            insts = bb.instructions
            new_list = []
            changed = False
            for inst in insts:
                si = inst.sync_info
                if si is not None and len(si.on_wait) > max_waits:
                    waits = list(si.on_wait)
                    extra, keep = waits[:-max_waits], waits[-max_waits:]
                    k = 0
                    while extra:
                        chunk, extra = extra[:max_waits], extra[max_waits:]
                        nop = mybir.InstNoOp(
                            name=f"{inst.name}-wsplit{k}", engine=inst.engine
                        )
                        nop.sync_info = mybir.SyncInfo(on_wait=chunk, on_update=[])
                        new_list.append(nop)
                        changed = True
                        k += 1
                    inst.sync_info = mybir.SyncInfo(
                        on_wait=keep, on_update=list(si.on_update)
                    )
                new_list.append(inst)
            if changed:
                bb.instructions = new_list


def build_nc():
    nc = bass.Bass()

    xT = nc.dram_tensor("xT", [KC, 128, T], BF16, kind="ExternalInput")
    wq = nc.dram_tensor("wq", [NPAIRS, 128, KC, 128], BF16, kind="ExternalInput")
    wk = nc.dram_tensor("wk", [NPAIRS, 128, KC, 128], BF16, kind="ExternalInput")
    wv = nc.dram_tensor("wv", [NPAIRS, 128, KC, 128], BF16, kind="ExternalInput")
    wo = nc.dram_tensor("wo", [NPAIRS, 128, C], BF16, kind="ExternalInput")
    # f16 output: halves HBM-write traffic; host sums partials in f32
    out = nc.dram_tensor("out", [NTB, 128, C], F16, kind="ExternalOutput")

    with tile.TileContext(nc) as tc:
        with (
            tc.tile_pool(name="const", bufs=1) as const,
            tc.tile_pool(name="persist", bufs=1) as persist,
            tc.tile_pool(name="temps", bufs=1) as temps,
            tc.tile_pool(name="attnp", bufs=6) as attnp,
            tc.tile_pool(name="normsb", bufs=3) as normsb,
            tc.tile_pool(name="outp", bufs=3) as outp,
            tc.tile_pool(name="flow", bufs=3, space="PSUM") as flow,
            tc.tile_pool(name="acc", bufs=2, space="PSUM") as acc,
        ):
            # Deferred-emission backlog: thunks emitted one per matmul-loop
            # iteration so latency-bound chains overlap dense matmul work.
            backlog = []

            def drain_one():
                if backlog:
                    backlog.pop(0)()

            def drain_all():
                while backlog:
                    backlog.pop(0)()

            # --- load inputs ----------------------------------------------
            wq_sb, wk_sb, wv_sb, wo_sb = [], [], [], []
            for p in range(NPAIRS):
                for lst, nm in ((wq_sb, "wq"), (wk_sb, "wk"), (wv_sb, "wv")):
                    lst.append(
                        persist.tile(
                            [128, KC, 128], BF16, tag=f"{nm}{p}", name=f"{nm}{p}"
                        )
                    )
                wo_sb.append(
                    persist.tile([128, C], BF16, tag=f"wo{p}", name=f"wo{p}")
                )
            # all input DMAs issued up front in consumption order (the DMA
            # engines serialize globally, so issue order ~= arrival order);
            # x split into column halves, wq0 split at its first chunk, so
            # the first projection matmul starts ~2.5us earlier
            x_sb = [
                persist.tile([128, T], BF16, tag=f"x{kc}", name=f"x{kc}")
                for kc in range(KC)
            ]
            loads = [
                (wq_sb[0][:, 0:1, :], wq[0][:, 0:1, :]),
                (x_sb[0][:, 0:1024], xT[0][:, 0:1024]),
                (wq_sb[0][:, 1:KC, :], wq[0][:, 1:KC, :]),
            ]
            for hh in range(2):
                for kc in range(KC):
                    if hh == 0 and kc == 0:
                        continue
                    loads.append(
                        (
                            x_sb[kc][:, hh * 1024 : (hh + 1) * 1024],
                            xT[kc][:, hh * 1024 : (hh + 1) * 1024],
                        )
                    )
                if hh == 0:
                    loads.insert(6, (wk_sb[0][:], wk[0]))
                    loads.append((wv_sb[0][:], wv[0]))
            loads += [
                (wo_sb[0][:], wo[0]),
                (wq_sb[1][:], wq[1]),
                (wk_sb[1][:], wk[1]),
                (wv_sb[1][:], wv[1]),
                (wo_sb[1][:], wo[1]),
            ]
            qrr = [nc.sync, nc.scalar]
            for i, (dst, src_ap) in enumerate(loads):
                qrr[i % 2].dma_start(dst, src_ap)

            # --- constants -------------------------------------------------
            ident = const.tile([128, 128], BF16, tag="ident", name="ident")
            make_identity(nc, ident[:])
            ident32 = const.tile([128, 128], F32, tag="ident32", name="ident32")
            make_identity(nc, ident32[:])
            # multiplicative causal mask for diagonal blocks of attnT:
            # valid where k_local <= q_local (upper triangle incl diag)
            tri = const.tile([128, 128], BF16, tag="tri", name="tri")
            make_upper_triangular(nc, tri[:], val=1.0, diag=True)


            # --- persistent per-pair tensors -------------------------------
            q_sb, k_sb, v_sb, yT_sb = [], [], [], []
            qpad, kpad = [], []  # per (pair, head): zero-padded to K=128
            for p in range(NPAIRS):
                q_sb.append(persist.tile([128, T], BF16, tag=f"qT{p}", name=f"qT{p}"))
                k_sb.append(persist.tile([128, T], BF16, tag=f"kT{p}", name=f"kT{p}"))
                v_sb.append(
                    persist.tile([128, NTB, 256], BF16, tag=f"v{p}", name=f"v{p}")
                )
                yT_sb.append(persist.tile([128, T], BF16, tag=f"yT{p}", name=f"yT{p}"))
                qpad.append(
                    [
                        persist.tile([128, T], BF16, tag=f"qp{p}{hd}", name=f"qp{p}{hd}")
                        for hd in range(2)
                    ]
                )
                kpad.append(
                    [
                        persist.tile([128, T], BF16, tag=f"kp{p}{hd}", name=f"kp{p}{hd}")
                        for hd in range(2)
                    ]
                )

            # --- projections ----------------------------------------------
            vt_tmps = [
                temps.tile([128, T], BF16, tag=f"vt{p}", name=f"vt{p}")
                for p in range(NPAIRS)
            ]

            def proj_chunk(p, w_sb, dst, hh):
                ps = flow.tile([128, 1024], F32, tag="flow", name="flow")
                for kc in range(KC):
                    for o2 in range(2):
                        nc.tensor.matmul(
                            ps[:, o2 * 512 : (o2 + 1) * 512],
                            w_sb[:, kc, :],
                            x_sb[kc][
                                :,
                                hh * 1024 + o2 * 512 : hh * 1024 + (o2 + 1) * 512,
                            ],
                            start=(kc == 0),
                            stop=(kc == KC - 1),
                        )
                # pair-0 copies run while ACT is idle; pair-1 copies are
                # emitted mid-attention where ACT is the bottleneck
                if p == 0:
                    nc.scalar.copy(dst[:, hh * 1024 : (hh + 1) * 1024], ps[:])
                else:
                    nc.vector.tensor_copy(
                        dst[:, hh * 1024 : (hh + 1) * 1024], ps[:]
                    )

            def make_pads(p):
                # zero-padded per-head copies of qT/kT (other head's rows = 0)
                # so the scores matmuls run with K=128 (HAM-visible)
                for hd in range(2):
                    beta = hd * 64
                    zlo, zhi = (64, 128) if hd == 0 else (0, 64)
                    for src_t, dst_t in (
                        (q_sb[p], qpad[p][hd]),
                        (k_sb[p], kpad[p][hd]),
                    ):
                        nc.gpsimd.memset(dst_t[zlo:zhi, :], 0.0)
                        nc.vector.tensor_copy(
                            dst_t[beta : beta + 64, :],
                            src_t[beta : beta + 64, :],
                        )

            def make_v(p):
                # v natural layout; 128-wide head blocks: v at 0:64, ones at
                # 64, zeros above
                nc.gpsimd.memset(v_sb[p][:], 0.0)
                nc.gpsimd.memset(
                    v_sb[p][:].rearrange("p tb (h e) -> p tb h e", h=2)[
                        :, :, :, 64:65
                    ],
                    1.0,
                )

            def v_tr(p, hd):
                # one big xbar transpose per head: [64, 2048] -> blocked
                # [128, 16, 64] (partition-wrapped transpose)
                nc.sync.dma_start_transpose(
                    v_sb[p][:, :, hd * 128 : hd * 128 + 64],
                    vt_tmps[p][hd * 64 : (hd + 1) * 64, :],
                )

            # pair 0 inline; pair 1 deferred into pair-0's attention loop so
            # its PE-dense projection matmuls overlap the exp backlog
            for hh in range(2):
                for w_sb, dst in (
                    (wq_sb[0], q_sb[0]),
                    (wk_sb[0], k_sb[0]),
                    (wv_sb[0], vt_tmps[0]),
                ):
                    proj_chunk(0, w_sb, dst, hh)
            make_pads(0)
            make_v(0)
            v_tr(0, 0)
            v_tr(0, 1)

            def mk(f, *a):
                return lambda: f(*a)

            for w_sb, dst in (
                (wq_sb[1], q_sb[1]),
                (wk_sb[1], k_sb[1]),
                (wv_sb[1], vt_tmps[1]),
            ):
                for hh in range(2):
                    backlog.append(mk(proj_chunk, 1, w_sb, dst, hh))
            backlog.append(mk(make_pads, 1))
            backlog.append(mk(make_v, 1))
            backlog.append(mk(v_tr, 1, 0))
            backlog.append(mk(v_tr, 1, 1))

            # --- attention -------------------------------------------------
            def norm_chain(p, Q, y_aug, act=False):
                """Emit thunks that normalize y_aug (divide rows 0:64 by the
                ones-row 64) into yT_sb[p][:, quarter Q]. Data transposes go
                through the DMA xbar (blocked 3D form); only the tiny [1,128]
                softmax-sum rows transpose on the PE (K=1, f32 precision)."""
                qlo = Q * 512
                st = {}

                def cp():
                    t = normsb.tile([65, 512], BF16, tag="ysb", name="ysb")
                    ts = normsb.tile([1, 512], F32, tag="srow", name="srow")
                    if act:  # tail: ACT is idle there
                        nc.scalar.copy(t[0:65, :], y_aug[0][0:65, :])
                        nc.scalar.copy(ts[:], y_aug[0][64:65, :])
                    else:
                        nc.vector.tensor_copy(t[0:65, :], y_aug[0][0:65, :])
                        nc.vector.tensor_copy(ts[:], y_aug[0][64:65, :])
                    st["ysb0"], st["srow0"] = t, ts

                def cp1():
                    t = normsb.tile([65, 512], BF16, tag="ysb1", name="ysb1")
                    ts = normsb.tile([1, 512], F32, tag="srow1", name="srow1")
                    if act:
                        nc.scalar.copy(t[0:65, :], y_aug[1][0:65, :])
                        nc.scalar.copy(ts[:], y_aug[1][64:65, :])
                    else:
                        nc.vector.tensor_copy(t[0:65, :], y_aug[1][0:65, :])
                        nc.vector.tensor_copy(ts[:], y_aug[1][64:65, :])
                    st["ysb1"], st["srow1"] = t, ts

                def fwd():
                    # blocked xbar transpose: [64, 512] -> [128, 4, 64]
                    for hd in range(2):
                        yn = normsb.tile(
                            [128, 4, 64], BF16, tag=f"ynat{hd}", name=f"ynat{hd}"
                        )
                        nc.sync.dma_start_transpose(
                            yn[:], st[f"ysb{hd}"][0:64, :]
                        )
                        st[f"ynat{hd}"] = yn
                    st["ynorm"] = normsb.tile(
                        [128, 4, 128], BF16, tag="ynorm", name="ynorm"
                    )

                def s_t(hd):
                    def f():
                        # transpose the four [1,128] sum rows into one PSUM
                        # tile, one reciprocal for all four
                        sps = flow.tile([128, 4], F32, tag="flow", name="flow")
                        for tb in range(4):
                            nc.tensor.transpose(
                                sps[:, tb : tb + 1],
                                st[f"srow{hd}"][:, tb * 128 : (tb + 1) * 128],
                                ident32[0:1, 0:1],
                            )
                        r = normsb.tile([128, 4], F32, tag=f"rcp{hd}",
                                        name=f"rcp{hd}")
                        nc.vector.reciprocal(r[:], sps[:])
                        st[f"r{hd}"] = r

                    return f

                def tb_step(tb):
                    def f():
                        for hd in range(2):
                            nc.vector.tensor_scalar_mul(
                                st["ynorm"][:, tb, hd * 64 : hd * 64 + 64],
                                st[f"ynat{hd}"][:, tb, :],
                                st[f"r{hd}"][:, tb : tb + 1],
                            )

                    return f

                def back(half):
                    # blocked xbar transpose per 256-col half so outproj
                    # thunks queued right behind wait on a smaller transfer
                    def f():
                        nc.sync.dma_start_transpose(
                            yT_sb[p][
                                :, qlo + half * 256 : qlo + (half + 1) * 256
                            ].rearrange("p (tb t) -> p tb t", tb=2),
                            st["ynorm"][:, 2 * half : 2 * half + 2, :].rearrange(
                                "p tb c -> p (tb c)"
                            ),
                        )

                    return f

                return [cp, cp1, fwd, s_t(0), s_t(1), tb_step(0), tb_step(1),
                        back(0), tb_step(2), tb_step(3), back(1)]

            def outproj(tb, act=False):
                def f():
                    o_sb = outp.tile([128, C], F16, tag="osb", name="osb")
                    for nch in range(2):
                        ps = flow.tile([128, 512], F32, tag="flow", name="flow")
                        for p in range(NPAIRS):
                            nc.tensor.matmul(
                                ps[:],
                                yT_sb[p][:, tb * 128 : (tb + 1) * 128],
                                wo_sb[p][:, nch * 512 : (nch + 1) * 512],
                                start=(p == 0),
                                stop=(p == NPAIRS - 1),
                            )
                        dst = o_sb[:, nch * 512 : (nch + 1) * 512]
                        if act or nch == 1:
                            nc.scalar.copy(dst, ps[:])
                        else:
                            nc.vector.tensor_copy(dst, ps[:])
                    nc.gpsimd.dma_start(out[tb], o_sb[:])

                return f

            # --- final-quarter (1,3) per-block norm+outproj ----------------
            # The last quarter's post-processing is the serial tail of the
            # whole kernel: do it per 128-col block (block b completes at AV
            # kb=12+b) with PE transposes instead of DMA-xbar ones, in two
            # stages so the cross-engine latency hides in attention iters.
            tailst = {}

            def tail_a(b, y_aug):
                st = {}
                ysb_t = normsb.tile([64, 256], F32, tag="tys", name="tys")
                srow_t = normsb.tile([1, 256], F32, tag="tsr", name="tsr")
                bl, bh = b * 128, (b + 1) * 128
                nc.vector.tensor_copy(ysb_t[:, 0:128], y_aug[0][0:64, bl:bh])
                nc.vector.tensor_copy(srow_t[:, 0:128], y_aug[0][64:65, bl:bh])
                nc.scalar.copy(ysb_t[:, 128:256], y_aug[1][0:64, bl:bh])
                nc.scalar.copy(srow_t[:, 128:256], y_aug[1][64:65, bl:bh])
                sps = flow.tile([128, 2], F32, tag="flow", name="tsps")
                for hd in range(2):
                    nc.tensor.transpose(
                        sps[:, hd : hd + 1],
                        srow_t[:, hd * 128 : (hd + 1) * 128],
                        ident32[0:1, 0:1],
                    )
                r = normsb.tile([128, 2], F32, tag="trcp", name="trcp")
                nc.vector.reciprocal(r[:], sps[:])
                ynat = flow.tile([128, 128], F32, tag="flow", name="tyn")
                for hd in range(2):
                    nc.tensor.transpose(
                        ynat[:, hd * 64 : (hd + 1) * 64],
                        ysb_t[:, hd * 128 : (hd + 1) * 128],
                        ident32[0:64, 0:64],
                    )
                st["ynat"], st["r"] = ynat, r
                tailst[b] = st

            def tail_b(b):
                st = tailst[b]
                ynat, r = st["ynat"], st["r"]
                tb = 12 + b
                q0 = 1536 + b * 128
                ynn = normsb.tile([128, 128], F32, tag="tynn", name="tynn")
                for hd in range(2):
                    nc.vector.tensor_scalar_mul(
                        ynn[:, hd * 64 : (hd + 1) * 64],
                        ynat[:, hd * 64 : (hd + 1) * 64],
                        r[:, hd : hd + 1],
                    )
                ytp = flow.tile([128, 128], F32, tag="flow", name="tytp")
                nc.tensor.transpose(ytp[:], ynn[:], ident32[:])
                nc.scalar.copy(yT_sb[1][:, q0 : q0 + 128], ytp[:])
                outproj(tb)()

            for p in range(NPAIRS):
                for Q in range(4):  # q quarters of 512
                    qlo = Q * 512
                    final = (p, Q) == (1, 3)
                    y_aug = [
                        acc.tile([128, 512], F32, tag="acc", name="acc")
                        for _ in range(2)
                    ]
                    nkb = 4 * Q + 4
                    # software pipeline: emit scores/exp for iteration i, then
                    # the av matmuls for iteration i-1, so the PE never waits
                    # on the exp of the tile it is about to consume.
                    # Each PSUM tile holds BOTH heads' score chunk (cols 0:512
                    # head A, 512:1024 head B) so one (strided) ACT exp
                    # covers them, halving ACT instruction-startup overhead.
                    pending = []

                    def flush_av(p=p, qlo=qlo, nkb=nkb, y_aug=y_aug,
                                 pending=pending):
                        for (kb, hd, at, q0, wdt) in pending:
                            nc.tensor.matmul(
                                y_aug[hd][:, q0 - qlo : q0 - qlo + wdt],
                                v_sb[p][:, kb, hd * 128 : (hd + 1) * 128],
                                at[:, hd * 512 : hd * 512 + wdt],
                                start=(kb == 0),
                                stop=(kb == nkb - 1),
                            )
                        pending.clear()

                    for kb in range(nkb):
                        qs = max(kb * 128, qlo)  # global q start
                        wdt = qlo + 512 - qs
                        ps = flow.tile([128, 1024], F32, tag="flow", name="flow")
                        for hd in range(2):
                            nc.tensor.matmul(
                                ps[:, hd * 512 : hd * 512 + wdt],
                                kpad[p][hd][:, kb * 128 : (kb + 1) * 128],
                                qpad[p][hd][:, qs : qs + wdt],
                                start=True,
                                stop=True,
                            )
                        flush_av()
                        if p == 1 and Q == 2 and 4 <= kb < 8:
                            # quarter (1,0)'s outprojs run inline here: its
                            # back-transposes landed a full quarter ago, so
                            # the in-order PE queue never blocks on them
                            outproj(kb - 4)()
                        if final and kb >= 8:
                            # (1,1)'s and (1,2)'s outprojs likewise (as early
                            # backlog thunks they head-of-line-stalled the PE
                            # ~4us waiting on in-flight back-transposes)
                            outproj(kb - 4)()
                        if final and kb >= 13:
                            # block kb-13 finished accumulating in that flush
                            if kb >= 14:
                                tail_b(kb - 14)
                            tail_a(kb - 13, y_aug)
                        drain_one()
                        if len(backlog) > 10:
                            drain_one()
                        if len(backlog) > 20:
                            drain_one()
                        at = attnp.tile([128, 1024], BF16, tag="attn",
                                        name="attn")
                        if wdt == 512:
                            nc.scalar.activation(
                                at[:], ps[:], mybir.ActivationFunctionType.Exp
                            )
                        else:
                            v2 = ps[:].rearrange("p (h w) -> p h w", h=2)
                            a2 = at[:].rearrange("p (h w) -> p h w", h=2)
                            nc.scalar.activation(
                                a2[:, :, 0:wdt],
                                v2[:, :, 0:wdt],
                                mybir.ActivationFunctionType.Exp,
                            )
                        if kb * 128 >= qlo:
                            # diagonal block: causal mask, both heads
                            for hd in range(2):
                                nc.vector.tensor_mul(
                                    at[:, hd * 512 : hd * 512 + 128],
                                    at[:, hd * 512 : hd * 512 + 128],
                                    tri[:],
                                )
                        for hd in range(2):
                            pending.append((kb, hd, at, qs, wdt))
                    flush_av()
                    if final:
                        tail_a(3, y_aug)
                        tail_b(2)
                        tail_b(3)
                    else:
                        backlog.extend(norm_chain(p, Q, y_aug))
            drain_all()

    split_waits(nc)
    nc.finalize()
    return nc


def _get_nc():
    global _nc_cache
    if _nc_cache is None:
        _nc_cache = build_nc()
    return _nc_cache


def _prep_core_inputs(x, Wq, Wk, Wv, Wo, core):
    b, g = core // 4, core % 4
    bf = ml_dtypes.bfloat16
    xT_host = np.ascontiguousarray(x[b].T).reshape(KC, 128, T).astype(bf)

    def pack_w(Wm, scale=1.0):
        # [NPAIRS, 128, KC, 128]: lhsT chunks; columns = 2 heads' channels
        outw = np.empty((NPAIRS, 128, KC, 128), dtype=bf)
        for p in range(NPAIRS):
            h0 = 4 * g + 2 * p
            blk = Wm[h0 * D : (h0 + 2) * D, :].T * scale  # [C, 128]
            outw[p] = blk.reshape(KC, 128, 128).transpose(1, 0, 2).astype(bf)
        return outw

    scale = 1.0 / np.sqrt(D)
    wo_host = np.empty((NPAIRS, 128, C), dtype=bf)
    for p in range(NPAIRS):
        h0 = 4 * g + 2 * p
        wo_host[p] = Wo[:, h0 * D : (h0 + 2) * D].T.astype(bf)
    return {
        "xT": xT_host,
        "wq": pack_w(Wq, scale),
        "wk": pack_w(Wk),
        "wv": pack_w(Wv),
        "wo": wo_host,
    }


def _ensure_ntff_hook():
    """antenv.axon_hooks is missing from this image; shim it and register
    the ctypes NTFF profile hook so trace=True captures profiles."""
    import types

    if "antenv.axon_hooks" in sys.modules:
        return
    mod = types.ModuleType("antenv.axon_hooks")
    mod._hook = None

    def set_axon_ntff_profile_hook(h):
        mod._hook = h

    def get_axon_ntff_profile_hook():
        return mod._hook

    mod.set_axon_ntff_profile_hook = set_axon_ntff_profile_hook
    mod.get_axon_ntff_profile_hook = get_axon_ntff_profile_hook
    sys.modules["antenv.axon_hooks"] = mod
    try:
        from trn_agent_boot.trn_boot import _ntff_profile_via_ctypes

        mod._hook = _ntff_profile_via_ctypes("/opt/axon/libaxon_pjrt.so")
    except Exception as e:
        print(f"ntff hook setup failed: {e}")


def kernel(x, Wq, Wk, Wv, Wo, _trace=False, _tracedir=None):
    if _trace:
        _ensure_ntff_hook()
    x = np.asarray(x, dtype=np.float32)
    Wq, Wk, Wv, Wo = (np.asarray(w, dtype=np.float32) for w in (Wq, Wk, Wv, Wo))
    nc = _get_nc()
    in_maps = [_prep_core_inputs(x, Wq, Wk, Wv, Wo, c) for c in range(NCORES)]
    res = run_bass_kernel_spmd(
        nc, in_maps, core_ids=list(range(NCORES)), trace=_trace, tmpdir=_tracedir
    )
    out = np.zeros((NB, T, C), dtype=np.float32)
    for c in range(NCORES):
        out[c // 4] += res.results[c]["out"].astype(np.float32).reshape(T, C)
    if _trace:
        kernel._last_results = res
    return out


# revision 32
# speedup vs baseline: 1.3827x; 1.0024x over previous
"""Causal self-attention (B=2, T=2048, C=1024, H=16, D=64) on 8 trn2 cores.

BASELINE RECONSTRUCTION (172968 ns) — restore to kernel.py if needed.

Sharding: data-parallel over B (2) x tensor-parallel over head groups (4).
Core c handles batch b = c // 4 and heads [4g, 4g+4) with g = c % 4.
Each core computes a partial output  y_local @ Wo_local.T  of shape [T, C];
the host sums the 4 partials per batch.
"""

import sys

sys.path.insert(0, "/opt/trn_rl_repo")

import numpy as np
import ml_dtypes

import concourse.bass as bass
import concourse.tile as tile
from concourse import mybir
from concourse.bass_utils import run_bass_kernel_spmd
from concourse.masks import make_identity, make_upper_triangular

BF16 = mybir.dt.bfloat16
F16 = mybir.dt.float16
F32 = mybir.dt.float32

T = 2048
C = 1024
H = 16
D = 64
NB = 2  # batch
NCORES = 8
NPAIRS = 2  # head pairs per core
KC = C // 128  # 8 contraction chunks for projections
NTB = T // 128  # 16 token blocks
HALF = T // 2  # 1024

_nc_cache = None


def split_waits(nc, max_waits=1):
    """This walrus build rejects instructions with more than one semaphore
    wait; move excess waits onto same-engine NOPs inserted just before."""
    for fn in nc.m.functions:
        for bb in fn.blocks:
            insts = bb.instructions
            new_list = []
            changed = False
            for inst in insts:
                si = inst.sync_info
                if si is not None and len(si.on_wait) > max_waits:
                    waits = list(si.on_wait)
                    extra, keep = waits[:-max_waits], waits[-max_waits:]
                    k = 0
                    while extra:
                        chunk, extra = extra[:max_waits], extra[max_waits:]
                        nop = mybir.InstNoOp(
                            name=f"{inst.name}-wsplit{k}", engine=inst.engine
                        )
                        nop.sync_info = mybir.SyncInfo(on_wait=chunk, on_update=[])
                        new_list.append(nop)
                        changed = True
                        k += 1
                    inst.sync_info = mybir.SyncInfo(
                        on_wait=keep, on_update=list(si.on_update)
                    )
                new_list.append(inst)
            if changed:
                bb.instructions = new_list


def build_nc():
    nc = bass.Bass()

    xT = nc.dram_tensor("xT", [KC, 128, T], BF16, kind="ExternalInput")
    wq = nc.dram_tensor("wq", [NPAIRS, 128, KC, 128], BF16, kind="ExternalInput")
    wk = nc.dram_tensor("wk", [NPAIRS, 128, KC, 128], BF16, kind="ExternalInput")
    wv = nc.dram_tensor("wv", [NPAIRS, 128, KC, 128], BF16, kind="ExternalInput")
    wo = nc.dram_tensor("wo", [NPAIRS, 128, C], BF16, kind="ExternalInput")
    # f16 output: halves HBM-write traffic; host sums partials in f32
    out = nc.dram_tensor("out", [NTB, 128, C], F16, kind="ExternalOutput")

    with tile.TileContext(nc) as tc:
        with (
            tc.tile_pool(name="const", bufs=1) as const,
            tc.tile_pool(name="persist", bufs=1) as persist,
            tc.tile_pool(name="temps", bufs=1) as temps,
            tc.tile_pool(name="attnp", bufs=6) as attnp,
            tc.tile_pool(name="normsb", bufs=3) as normsb,
            tc.tile_pool(name="outp", bufs=3) as outp,
            tc.tile_pool(name="flow", bufs=3, space="PSUM") as flow,
            tc.tile_pool(name="acc", bufs=2, space="PSUM") as acc,
        ):
            # Deferred-emission backlog: thunks emitted one per matmul-loop
            # iteration so latency-bound chains overlap dense matmul work.
            backlog = []

            def drain_one():
                if backlog:
                    backlog.pop(0)()

            def drain_all():
                while backlog:
                    backlog.pop(0)()

            # --- load inputs ----------------------------------------------
            wq_sb, wk_sb, wv_sb, wo_sb = [], [], [], []
            for p in range(NPAIRS):
                for lst, nm in ((wq_sb, "wq"), (wk_sb, "wk"), (wv_sb, "wv")):
                    lst.append(
                        persist.tile(
                            [128, KC, 128], BF16, tag=f"{nm}{p}", name=f"{nm}{p}"
                        )
                    )
                wo_sb.append(
                    persist.tile([128, C], BF16, tag=f"wo{p}", name=f"wo{p}")
                )
            # all input DMAs issued up front in consumption order (the DMA
            # engines serialize globally, so issue order ~= arrival order);
            # x split into column halves, wq0 split at its first chunk, so
            # the first projection matmul starts ~2.5us earlier
            x_sb = [
                persist.tile([128, T], BF16, tag=f"x{kc}", name=f"x{kc}")
                for kc in range(KC)
            ]
            loads = [
                (wq_sb[0][:, 0:1, :], wq[0][:, 0:1, :]),
                (x_sb[0][:, 0:1024], xT[0][:, 0:1024]),
                (wq_sb[0][:, 1:KC, :], wq[0][:, 1:KC, :]),
            ]
            for hh in range(2):
                for kc in range(KC):
                    if hh == 0 and kc == 0:
                        continue
                    loads.append(
                        (
                            x_sb[kc][:, hh * 1024 : (hh + 1) * 1024],
                            xT[kc][:, hh * 1024 : (hh + 1) * 1024],
                        )
                    )
                if hh == 0:
                    loads.insert(6, (wk_sb[0][:], wk[0]))
                    loads.append((wv_sb[0][:], wv[0]))
            loads += [
                (wo_sb[0][:], wo[0]),
                (wq_sb[1][:], wq[1]),
                (wk_sb[1][:], wk[1]),
                (wv_sb[1][:], wv[1]),
                (wo_sb[1][:], wo[1]),
            ]
            qrr = [nc.sync, nc.scalar]
            for i, (dst, src_ap) in enumerate(loads):
                qrr[i % 2].dma_start(dst, src_ap)

            # --- constants -------------------------------------------------
            ident = const.tile([128, 128], BF16, tag="ident", name="ident")
            make_identity(nc, ident[:])
            ident32 = const.tile([128, 128], F32, tag="ident32", name="ident32")
            make_identity(nc, ident32[:])
            # multiplicative causal mask for diagonal blocks of attnT:
            # valid where k_local <= q_local (upper triangle incl diag)
            tri = const.tile([128, 128], BF16, tag="tri", name="tri")
            make_upper_triangular(nc, tri[:], val=1.0, diag=True)


            # --- persistent per-pair tensors -------------------------------
            q_sb, k_sb, v_sb, yT_sb = [], [], [], []
            qpad, kpad = [], []  # per (pair, head): zero-padded to K=128
            for p in range(NPAIRS):
                q_sb.append(persist.tile([128, T], BF16, tag=f"qT{p}", name=f"qT{p}"))
                k_sb.append(persist.tile([128, T], BF16, tag=f"kT{p}", name=f"kT{p}"))
                v_sb.append(
                    persist.tile([128, NTB, 256], BF16, tag=f"v{p}", name=f"v{p}")
                )
                yT_sb.append(persist.tile([128, T], BF16, tag=f"yT{p}", name=f"yT{p}"))
                kpad.append(
                    [
                        persist.tile([128, T], BF16, tag=f"kp{p}{hd}", name=f"kp{p}{hd}")
                        for hd in range(2)
                    ]
                )

            # --- projections ----------------------------------------------
            vt_tmps = [
                temps.tile([128, T], BF16, tag=f"vt{p}", name=f"vt{p}")
                for p in range(NPAIRS)
            ]

            def proj_chunk(p, w_sb, dst, hh):
                ps = flow.tile([128, 1024], F32, tag="flow", name="flow")
                for kc in range(KC):
                    for o2 in range(2):
                        nc.tensor.matmul(
                            ps[:, o2 * 512 : (o2 + 1) * 512],
                            w_sb[:, kc, :],
                            x_sb[kc][
                                :,
                                hh * 1024 + o2 * 512 : hh * 1024 + (o2 + 1) * 512,
                            ],
                            start=(kc == 0),
                            stop=(kc == KC - 1),
                        )
                # pair-0 copies run while ACT is idle; pair-1 copies are
                # emitted mid-attention where ACT is the bottleneck
                if p == 0:
                    nc.scalar.copy(dst[:, hh * 1024 : (hh + 1) * 1024], ps[:])
                else:
                    nc.vector.tensor_copy(
                        dst[:, hh * 1024 : (hh + 1) * 1024], ps[:]
                    )

            def make_pads(p):
                # zero-padded per-head copies of kT (other head's rows = 0)
                # so the scores matmuls run with K=128 (HAM-visible).  qT
                # needs no padding: kpad's zero rows nullify the other
                # head's q rows in the contraction.
                for hd in range(2):
                    beta = hd * 64
                    zlo, zhi = (64, 128) if hd == 0 else (0, 64)
                    dst_t = kpad[p][hd]
                    nc.gpsimd.memset(dst_t[zlo:zhi, :], 0.0)
                    nc.vector.tensor_copy(
                        dst_t[beta : beta + 64, :],
                        k_sb[p][beta : beta + 64, :],
                    )

            def make_v(p):
                # v natural layout; 128-wide head blocks: v at 0:64, ones at
                # 64, zeros above
                nc.gpsimd.memset(v_sb[p][:], 0.0)
                nc.gpsimd.memset(
                    v_sb[p][:].rearrange("p tb (h e) -> p tb h e", h=2)[
                        :, :, :, 64:65
                    ],
                    1.0,
                )

            def v_tr(p, hd):
                # one big xbar transpose per head: [64, 2048] -> blocked
                # [128, 16, 64] (partition-wrapped transpose)
                nc.sync.dma_start_transpose(
                    v_sb[p][:, :, hd * 128 : hd * 128 + 64],
                    vt_tmps[p][hd * 64 : (hd + 1) * 64, :],
                )

            # pair 0 inline; pair 1 deferred into pair-0's attention loop so
            # its PE-dense projection matmuls overlap the exp backlog
            for hh in range(2):
                for w_sb, dst in (
                    (wq_sb[0], q_sb[0]),
                    (wk_sb[0], k_sb[0]),
                    (wv_sb[0], vt_tmps[0]),
                ):
                    proj_chunk(0, w_sb, dst, hh)
            make_pads(0)
            make_v(0)
            v_tr(0, 0)
            v_tr(0, 1)

            def mk(f, *a):
                return lambda: f(*a)

            for w_sb, dst in (
                (wq_sb[1], q_sb[1]),
                (wk_sb[1], k_sb[1]),
                (wv_sb[1], vt_tmps[1]),
            ):
                for hh in range(2):
                    backlog.append(mk(proj_chunk, 1, w_sb, dst, hh))
            backlog.append(mk(make_pads, 1))
            backlog.append(mk(make_v, 1))
            backlog.append(mk(v_tr, 1, 0))
            backlog.append(mk(v_tr, 1, 1))

            # --- attention -------------------------------------------------
            def norm_chain(p, Q, y_aug, act=False):
                """Emit thunks that normalize y_aug (divide rows 0:64 by the
                ones-row 64) into yT_sb[p][:, quarter Q]. Data transposes go
                through the DMA xbar (blocked 3D form); only the tiny [1,128]
                softmax-sum rows transpose on the PE (K=1, f32 precision)."""
                qlo = Q * 512
                st = {}

                def cp():
                    t = normsb.tile([65, 512], BF16, tag="ysb", name="ysb")
                    ts = normsb.tile([1, 512], F32, tag="srow", name="srow")
                    if act:  # tail: ACT is idle there
                        nc.scalar.copy(t[0:65, :], y_aug[0][0:65, :])
                        nc.scalar.copy(ts[:], y_aug[0][64:65, :])
                    else:
                        nc.vector.tensor_copy(t[0:65, :], y_aug[0][0:65, :])
                        nc.vector.tensor_copy(ts[:], y_aug[0][64:65, :])
                    st["ysb0"], st["srow0"] = t, ts

                def cp1():
                    t = normsb.tile([65, 512], BF16, tag="ysb1", name="ysb1")
                    ts = normsb.tile([1, 512], F32, tag="srow1", name="srow1")
                    if act:
                        nc.scalar.copy(t[0:65, :], y_aug[1][0:65, :])
                        nc.scalar.copy(ts[:], y_aug[1][64:65, :])
                    else:
                        nc.vector.tensor_copy(t[0:65, :], y_aug[1][0:65, :])
                        nc.vector.tensor_copy(ts[:], y_aug[1][64:65, :])
                    st["ysb1"], st["srow1"] = t, ts

                def fwd():
                    # blocked xbar transpose: [64, 512] -> [128, 4, 64]
                    for hd in range(2):
                        yn = normsb.tile(
                            [128, 4, 64], BF16, tag=f"ynat{hd}", name=f"ynat{hd}"
                        )
                        nc.sync.dma_start_transpose(
                            yn[:], st[f"ysb{hd}"][0:64, :]
                        )
                        st[f"ynat{hd}"] = yn
                    st["ynorm"] = normsb.tile(
                        [128, 4, 128], BF16, tag="ynorm", name="ynorm"
                    )

                def s_t(hd):
                    def f():
                        # transpose the four [1,128] sum rows into one PSUM
                        # tile, one reciprocal for all four
                        sps = flow.tile([128, 4], F32, tag="flow", name="flow")
                        for tb in range(4):
                            nc.tensor.transpose(
                                sps[:, tb : tb + 1],
                                st[f"srow{hd}"][:, tb * 128 : (tb + 1) * 128],
                                ident32[0:1, 0:1],
                            )
                        r = normsb.tile([128, 4], F32, tag=f"rcp{hd}",
                                        name=f"rcp{hd}")
                        nc.vector.reciprocal(r[:], sps[:])
                        st[f"r{hd}"] = r

                    return f

                def tb_step(tb):
                    def f():
                        for hd in range(2):
                            nc.vector.tensor_scalar_mul(
                                st["ynorm"][:, tb, hd * 64 : hd * 64 + 64],
                                st[f"ynat{hd}"][:, tb, :],
                                st[f"r{hd}"][:, tb : tb + 1],
                            )

                    return f

                def back(half):
                    # blocked xbar transpose per 256-col half so outproj
                    # thunks queued right behind wait on a smaller transfer
                    def f():
                        nc.sync.dma_start_transpose(
                            yT_sb[p][
                                :, qlo + half * 256 : qlo + (half + 1) * 256
                            ].rearrange("p (tb t) -> p tb t", tb=2),
                            st["ynorm"][:, 2 * half : 2 * half + 2, :].rearrange(
                                "p tb c -> p (tb c)"
                            ),
                        )

                    return f

                return [cp, cp1, fwd, s_t(0), s_t(1), tb_step(0), tb_step(1),
                        back(0), tb_step(2), tb_step(3), back(1)]

            def outproj(tb, act=False):
                def f():
                    o_sb = outp.tile([128, C], F16, tag="osb", name="osb")
                    for nch in range(2):
                        ps = flow.tile([128, 512], F32, tag="flow", name="flow")
                        for p in range(NPAIRS):
                            nc.tensor.matmul(
                                ps[:],
                                yT_sb[p][:, tb * 128 : (tb + 1) * 128],
                                wo_sb[p][:, nch * 512 : (nch + 1) * 512],
                                start=(p == 0),
                                stop=(p == NPAIRS - 1),
                            )
                        dst = o_sb[:, nch * 512 : (nch + 1) * 512]
                        if act:
                            # tail: ACT is idle after the last exp
                            nc.scalar.copy(dst, ps[:])
                        else:
                            # in-attention: ACT is exp-bound, keep it clear
                            nc.vector.tensor_copy(dst, ps[:])
                    nc.gpsimd.dma_start(out[tb], o_sb[:])

                return f

            # --- final-quarter (1,3) per-block norm+outproj ----------------
            # The last quarter's post-processing is the serial tail of the
            # whole kernel: do it per 128-col block (block b completes at AV
            # kb=12+b) with PE transposes instead of DMA-xbar ones, in two
            # stages so the cross-engine latency hides in attention iters.
            tailst = {}

            def tail_a(b, y_aug):
                st = {}
                ysb_t = normsb.tile([64, 256], F32, tag="tys", name="tys")
                srow_t = normsb.tile([1, 256], F32, tag="tsr", name="tsr")
                bl, bh = b * 128, (b + 1) * 128
                nc.vector.tensor_copy(ysb_t[:, 0:128], y_aug[0][0:64, bl:bh])
                nc.vector.tensor_copy(srow_t[:, 0:128], y_aug[0][64:65, bl:bh])
                nc.scalar.copy(ysb_t[:, 128:256], y_aug[1][0:64, bl:bh])
                nc.scalar.copy(srow_t[:, 128:256], y_aug[1][64:65, bl:bh])
                sps = flow.tile([128, 2], F32, tag="flow", name="tsps")
                for hd in range(2):
                    nc.tensor.transpose(
                        sps[:, hd : hd + 1],
                        srow_t[:, hd * 128 : (hd + 1) * 128],
                        ident32[0:1, 0:1],
                    )
                r = normsb.tile([128, 2], F32, tag="trcp", name="trcp")
                nc.vector.reciprocal(r[:], sps[:])
                ynat = flow.tile([128, 128], F32, tag="flow", name="tyn")
                for hd in range(2):
                    nc.tensor.transpose(
                        ynat[:, hd * 64 : (hd + 1) * 64],
                        ysb_t[:, hd * 128 : (hd + 1) * 128],
                        ident32[0:64, 0:64],
                    )
                st["ynat"], st["r"] = ynat, r
                tailst[b] = st

            def tail_b(b):
                st = tailst[b]
                ynat, r = st["ynat"], st["r"]
                tb = 12 + b
                q0 = 1536 + b * 128
                ynn = normsb.tile([128, 128], F32, tag="tynn", name="tynn")
                for hd in range(2):
                    nc.vector.tensor_scalar_mul(
                        ynn[:, hd * 64 : (hd + 1) * 64],
                        ynat[:, hd * 64 : (hd + 1) * 64],
                        r[:, hd : hd + 1],
                    )
                ytp = flow.tile([128, 128], F32, tag="flow", name="tytp")
                nc.tensor.transpose(ytp[:], ynn[:], ident32[:])
                nc.scalar.copy(yT_sb[1][:, q0 : q0 + 128], ytp[:])
                outproj(tb)()

            for p in range(NPAIRS):
                for Q in range(4):  # q quarters of 512
                    qlo = Q * 512
                    final = (p, Q) == (1, 3)
                    y_aug = [
                        acc.tile([128, 512], F32, tag="acc", name="acc")
                        for _ in range(2)
                    ]
                    nkb = 4 * Q + 4
                    # software pipeline: emit scores/exp for iteration i, then
                    # the av matmuls for iteration i-1, so the PE never waits
                    # on the exp of the tile it is about to consume.
                    # Each PSUM tile holds BOTH heads' score chunk (cols 0:512
                    # head A, 512:1024 head B) so one (strided) ACT exp
                    # covers them, halving ACT instruction-startup overhead.
                    pending = []

                    def flush_av(p=p, qlo=qlo, nkb=nkb, y_aug=y_aug,
                                 pending=pending):
                        for (kb, hd, at, q0, wdt) in pending:
                            nc.tensor.matmul(
                                y_aug[hd][:, q0 - qlo : q0 - qlo + wdt],
                                v_sb[p][:, kb, hd * 128 : (hd + 1) * 128],
                                at[:, hd * 512 : hd * 512 + wdt],
                                start=(kb == 0),
                                stop=(kb == nkb - 1),
                            )
                        pending.clear()

                    for kb in range(nkb):
                        qs = max(kb * 128, qlo)  # global q start
                        wdt = qlo + 512 - qs
                        ps = flow.tile([128, 1024], F32, tag="flow", name="flow")
                        for hd in range(2):
                            nc.tensor.matmul(
                                ps[:, hd * 512 : hd * 512 + wdt],
                                kpad[p][hd][:, kb * 128 : (kb + 1) * 128],
                                q_sb[p][:, qs : qs + wdt],
                                start=True,
                                stop=True,
                            )
                        flush_av()
                        if p == 1 and Q == 2 and 4 <= kb < 8:
                            # quarter (1,0)'s outprojs run inline here: its
                            # back-transposes landed a full quarter ago, so
                            # the in-order PE queue never blocks on them
                            outproj(kb - 4)()
                        if final and kb >= 8:
                            # (1,1)'s and (1,2)'s outprojs likewise (as early
                            # backlog thunks they head-of-line-stalled the PE
                            # ~4us waiting on in-flight back-transposes)
                            outproj(kb - 4)()
                        if final and kb >= 13:
                            # block kb-13 finished accumulating in that flush
                            if kb >= 14:
                                tail_b(kb - 14)
                            tail_a(kb - 13, y_aug)
                        drain_one()
                        if len(backlog) > 10:
                            drain_one()
                        if len(backlog) > 20:
                            drain_one()
                        at = attnp.tile([128, 1024], BF16, tag="attn",
                                        name="attn")
                        if wdt == 512:
                            nc.scalar.activation(
                                at[:], ps[:], mybir.ActivationFunctionType.Exp
                            )
                        else:
                            v2 = ps[:].rearrange("p (h w) -> p h w", h=2)
                            a2 = at[:].rearrange("p (h w) -> p h w", h=2)
                            nc.scalar.activation(
                                a2[:, :, 0:wdt],
                                v2[:, :, 0:wdt],
                                mybir.ActivationFunctionType.Exp,
                            )
                        if kb * 128 >= qlo:
                            # diagonal block: causal mask, both heads
                            for hd in range(2):
                                nc.vector.tensor_mul(
                                    at[:, hd * 512 : hd * 512 + 128],
                                    at[:, hd * 512 : hd * 512 + 128],
                                    tri[:],
                                )
                        for hd in range(2):
                            pending.append((kb, hd, at, qs, wdt))
                    flush_av()
                    if final:
                        tail_a(3, y_aug)
                        tail_b(2)
                        tail_b(3)
                    else:
                        backlog.extend(norm_chain(p, Q, y_aug))
            drain_all()

    split_waits(nc)
    nc.finalize()
    return nc


def _get_nc():
    global _nc_cache
    if _nc_cache is None:
        _nc_cache = build_nc()
    return _nc_cache


def _prep_core_inputs(x, Wq, Wk, Wv, Wo, core):
    b, g = core // 4, core % 4
    bf = ml_dtypes.bfloat16
    xT_host = np.ascontiguousarray(x[b].T).reshape(KC, 128, T).astype(bf)

    def pack_w(Wm, scale=1.0):
        # [NPAIRS, 128, KC, 128]: lhsT chunks; columns = 2 heads' channels
        outw = np.empty((NPAIRS, 128, KC, 128), dtype=bf)
        for p in range(NPAIRS):
            h0 = 4 * g + 2 * p
            blk = Wm[h0 * D : (h0 + 2) * D, :].T * scale  # [C, 128]
            outw[p] = blk.reshape(KC, 128, 128).transpose(1, 0, 2).astype(bf)
        return outw

    scale = 1.0 / np.sqrt(D)
    wo_host = np.empty((NPAIRS, 128, C), dtype=bf)
    for p in range(NPAIRS):
        h0 = 4 * g + 2 * p
        wo_host[p] = Wo[:, h0 * D : (h0 + 2) * D].T.astype(bf)
    return {
        "xT": xT_host,
        "wq": pack_w(Wq, scale),
        "wk": pack_w(Wk),
        "wv": pack_w(Wv),
        "wo": wo_host,
    }


def _ensure_ntff_hook():
    """antenv.axon_hooks is missing from this image; shim it and register
    the ctypes NTFF profile hook so trace=True captures profiles."""
    import types

    if "antenv.axon_hooks" in sys.modules:
        return
    mod = types.ModuleType("antenv.axon_hooks")
    mod._hook = None

    def set_axon_ntff_profile_hook(h):
        mod._hook = h

    def get_axon_ntff_profile_hook():
        return mod._hook

    mod.set_axon_ntff_profile_hook = set_axon_ntff_profile_hook
    mod.get_axon_ntff_profile_hook = get_axon_ntff_profile_hook
    sys.modules["antenv.axon_hooks"] = mod
    try:
        from trn_agent_boot.trn_boot import _ntff_profile_via_ctypes

        mod._hook = _ntff_profile_via_ctypes("/opt/axon/libaxon_pjrt.so")
    except Exception as e:
        print(f"ntff hook setup failed: {e}")


def kernel(x, Wq, Wk, Wv, Wo, _trace=False, _tracedir=None):
    if _trace:
        _ensure_ntff_hook()
    x = np.asarray(x, dtype=np.float32)
    Wq, Wk, Wv, Wo = (np.asarray(w, dtype=np.float32) for w in (Wq, Wk, Wv, Wo))
    nc = _get_nc()
    in_maps = [_prep_core_inputs(x, Wq, Wk, Wv, Wo, c) for c in range(NCORES)]
    res = run_bass_kernel_spmd(
        nc, in_maps, core_ids=list(range(NCORES)), trace=_trace, tmpdir=_tracedir
    )
    out = np.zeros((NB, T, C), dtype=np.float32)
    for c in range(NCORES):
        out[c // 4] += res.results[c]["out"].astype(np.float32).reshape(T, C)
    if _trace:
        kernel._last_results = res
    return out


# revision 34
# speedup vs baseline: 1.3992x; 1.0119x over previous
"""Causal self-attention (B=2, T=2048, C=1024, H=16, D=64) on 8 trn2 cores.

BASELINE RECONSTRUCTION (172968 ns) — restore to kernel.py if needed.

Sharding: data-parallel over B (2) x tensor-parallel over head groups (4).
Core c handles batch b = c // 4 and heads [4g, 4g+4) with g = c % 4.
Each core computes a partial output  y_local @ Wo_local.T  of shape [T, C];
the host sums the 4 partials per batch.
"""

import sys

sys.path.insert(0, "/opt/trn_rl_repo")

import numpy as np
import ml_dtypes

import concourse.bass as bass
import concourse.tile as tile
from concourse import mybir
from concourse.bass_utils import run_bass_kernel_spmd
from concourse.masks import make_identity, make_upper_triangular

BF16 = mybir.dt.bfloat16
F16 = mybir.dt.float16
F32 = mybir.dt.float32

T = 2048
C = 1024
H = 16
D = 64
NB = 2  # batch
NCORES = 8
NPAIRS = 2  # head pairs per core
KC = C // 128  # 8 contraction chunks for projections
NTB = T // 128  # 16 token blocks
HALF = T // 2  # 1024

_nc_cache = None


def split_waits(nc, max_waits=1):
    """This walrus build rejects instructions with more than one semaphore
    wait; move excess waits onto same-engine NOPs inserted just before."""
    for fn in nc.m.functions:
        for bb in fn.blocks:
            insts = bb.instructions
            new_list = []
            changed = False
            for inst in insts:
                si = inst.sync_info
                if si is not None and len(si.on_wait) > max_waits:
                    waits = list(si.on_wait)
                    extra, keep = waits[:-max_waits], waits[-max_waits:]
                    k = 0
                    while extra:
                        chunk, extra = extra[:max_waits], extra[max_waits:]
                        nop = mybir.InstNoOp(
                            name=f"{inst.name}-wsplit{k}", engine=inst.engine
                        )
                        nop.sync_info = mybir.SyncInfo(on_wait=chunk, on_update=[])
                        new_list.append(nop)
                        changed = True
                        k += 1
                    inst.sync_info = mybir.SyncInfo(
                        on_wait=keep, on_update=list(si.on_update)
                    )
                new_list.append(inst)
            if changed:
                bb.instructions = new_list


def build_nc():
    nc = bass.Bass()

    xT = nc.dram_tensor("xT", [KC, 128, T], BF16, kind="ExternalInput")
    wq = nc.dram_tensor("wq", [NPAIRS, 128, KC, 128], BF16, kind="ExternalInput")
    wk = nc.dram_tensor("wk", [NPAIRS, 128, KC, 128], BF16, kind="ExternalInput")
    wv = nc.dram_tensor("wv", [NPAIRS, 128, KC, 128], BF16, kind="ExternalInput")
    wo = nc.dram_tensor("wo", [NPAIRS, 128, C], BF16, kind="ExternalInput")
    # f16 output: halves HBM-write traffic; host sums partials in f32
    out = nc.dram_tensor("out", [NTB, 128, C], F16, kind="ExternalOutput")

    with tile.TileContext(nc) as tc:
        with (
            tc.tile_pool(name="const", bufs=1) as const,
            tc.tile_pool(name="persist", bufs=1) as persist,
            tc.tile_pool(name="temps", bufs=1) as temps,
            tc.tile_pool(name="attnp", bufs=6) as attnp,
            tc.tile_pool(name="normsb", bufs=3) as normsb,
            tc.tile_pool(name="outp", bufs=3) as outp,
            tc.tile_pool(name="flow", bufs=3, space="PSUM") as flow,
            tc.tile_pool(name="acc", bufs=2, space="PSUM") as acc,
        ):
            # Deferred-emission backlog: thunks emitted one per matmul-loop
            # iteration so latency-bound chains overlap dense matmul work.
            backlog = []

            def drain_one():
                if backlog:
                    backlog.pop(0)()

            def drain_all():
                while backlog:
                    backlog.pop(0)()

            # --- load inputs ----------------------------------------------
            wq_sb, wk_sb, wv_sb, wo_sb = [], [], [], []
            for p in range(NPAIRS):
                for lst, nm in ((wq_sb, "wq"), (wk_sb, "wk"), (wv_sb, "wv")):
                    lst.append(
                        persist.tile(
                            [128, KC, 128], BF16, tag=f"{nm}{p}", name=f"{nm}{p}"
                        )
                    )
                wo_sb.append(
                    persist.tile([128, C], BF16, tag=f"wo{p}", name=f"wo{p}")
                )
            # all input DMAs issued up front in consumption order (the DMA
            # engines serialize globally, so issue order ~= arrival order);
            # x split into column halves, wq0 split at its first chunk, so
            # the first projection matmul starts ~2.5us earlier
            x_sb = [
                persist.tile([128, T], BF16, tag=f"x{kc}", name=f"x{kc}")
                for kc in range(KC)
            ]
            loads = [
                (wq_sb[0][:, 0:1, :], wq[0][:, 0:1, :]),
                (x_sb[0][:, 0:1024], xT[0][:, 0:1024]),
                (wq_sb[0][:, 1:KC, :], wq[0][:, 1:KC, :]),
            ]
            for hh in range(2):
                for kc in range(KC):
                    if hh == 0 and kc == 0:
                        continue
                    loads.append(
                        (
                            x_sb[kc][:, hh * 1024 : (hh + 1) * 1024],
                            xT[kc][:, hh * 1024 : (hh + 1) * 1024],
                        )
                    )
                if hh == 0:
                    loads.insert(6, (wk_sb[0][:], wk[0]))
                    loads.append((wv_sb[0][:], wv[0]))
            loads += [
                (wo_sb[0][:], wo[0]),
                (wq_sb[1][:], wq[1]),
                (wk_sb[1][:], wk[1]),
                (wv_sb[1][:], wv[1]),
                (wo_sb[1][:], wo[1]),
            ]
            qrr = [nc.sync, nc.scalar]
            for i, (dst, src_ap) in enumerate(loads):
                qrr[i % 2].dma_start(dst, src_ap)

            # --- constants -------------------------------------------------
            ident = const.tile([128, 128], BF16, tag="ident", name="ident")
            make_identity(nc, ident[:])
            ident32 = const.tile([128, 128], F32, tag="ident32", name="ident32")
            make_identity(nc, ident32[:])
            # multiplicative causal mask for diagonal blocks of attnT:
            # valid where k_local <= q_local (upper triangle incl diag)
            tri = const.tile([128, 128], BF16, tag="tri", name="tri")
            make_upper_triangular(nc, tri[:], val=1.0, diag=True)


            # --- persistent per-pair tensors -------------------------------
            q_sb, k_sb, v_sb, yT_sb = [], [], [], []
            qpad, kpad = [], []  # per (pair, head): zero-padded to K=128
            for p in range(NPAIRS):
                q_sb.append(persist.tile([128, T], BF16, tag=f"qT{p}", name=f"qT{p}"))
                k_sb.append(persist.tile([128, T], BF16, tag=f"kT{p}", name=f"kT{p}"))
                v_sb.append(
                    persist.tile([128, NTB, 256], BF16, tag=f"v{p}", name=f"v{p}")
                )
                yT_sb.append(persist.tile([128, T], BF16, tag=f"yT{p}", name=f"yT{p}"))
                kpad.append(
                    [
                        persist.tile([128, T], BF16, tag=f"kp{p}{hd}", name=f"kp{p}{hd}")
                        for hd in range(2)
                    ]
                )

            # --- projections ----------------------------------------------
            vt_tmps = [
                temps.tile([128, T], BF16, tag=f"vt{p}", name=f"vt{p}")
                for p in range(NPAIRS)
            ]

            def proj_chunk(p, w_sb, dst, hh):
                ps = flow.tile([128, 1024], F32, tag="flow", name="flow")
                for kc in range(KC):
                    for o2 in range(2):
                        nc.tensor.matmul(
                            ps[:, o2 * 512 : (o2 + 1) * 512],
                            w_sb[:, kc, :],
                            x_sb[kc][
                                :,
                                hh * 1024 + o2 * 512 : hh * 1024 + (o2 + 1) * 512,
                            ],
                            start=(kc == 0),
                            stop=(kc == KC - 1),
                        )
                # pair-0 copies run while ACT is idle; pair-1 copies are
                # emitted mid-attention where ACT is the bottleneck
                if p == 0:
                    nc.scalar.copy(dst[:, hh * 1024 : (hh + 1) * 1024], ps[:])
                else:
                    nc.vector.tensor_copy(
                        dst[:, hh * 1024 : (hh + 1) * 1024], ps[:]
                    )

            def make_pads(p):
                # zero-padded per-head copies of kT (other head's rows = 0)
                # so the scores matmuls run with K=128 (HAM-visible).  qT
                # needs no padding: kpad's zero rows nullify the other
                # head's q rows in the contraction.
                for hd in range(2):
                    beta = hd * 64
                    zlo, zhi = (64, 128) if hd == 0 else (0, 64)
                    dst_t = kpad[p][hd]
                    nc.gpsimd.memset(dst_t[zlo:zhi, :], 0.0)
                    nc.vector.tensor_copy(
                        dst_t[beta : beta + 64, :],
                        k_sb[p][beta : beta + 64, :],
                    )

            def make_v(p):
                # v natural layout; 128-wide head blocks: v at 0:64, ones at
                # 64, zeros above
                nc.gpsimd.memset(v_sb[p][:], 0.0)
                nc.gpsimd.memset(
                    v_sb[p][:].rearrange("p tb (h e) -> p tb h e", h=2)[
                        :, :, :, 64:65
                    ],
                    1.0,
                )

            def v_tr(p, hd):
                # one big xbar transpose per head: [64, 2048] -> blocked
                # [128, 16, 64] (partition-wrapped transpose)
                nc.sync.dma_start_transpose(
                    v_sb[p][:, :, hd * 128 : hd * 128 + 64],
                    vt_tmps[p][hd * 64 : (hd + 1) * 64, :],
                )

            # pair 0 inline; pair 1 deferred into pair-0's attention loop so
            # its PE-dense projection matmuls overlap the exp backlog
            for hh in range(2):
                for w_sb, dst in (
                    (wq_sb[0], q_sb[0]),
                    (wk_sb[0], k_sb[0]),
                    (wv_sb[0], vt_tmps[0]),
                ):
                    proj_chunk(0, w_sb, dst, hh)
            make_pads(0)
            make_v(0)
            v_tr(0, 0)
            v_tr(0, 1)

            def mk(f, *a):
                return lambda: f(*a)

            for w_sb, dst in (
                (wq_sb[1], q_sb[1]),
                (wk_sb[1], k_sb[1]),
                (wv_sb[1], vt_tmps[1]),
            ):
                for hh in range(2):
                    backlog.append(mk(proj_chunk, 1, w_sb, dst, hh))
            backlog.append(mk(make_pads, 1))
            backlog.append(mk(make_v, 1))
            backlog.append(mk(v_tr, 1, 0))
            backlog.append(mk(v_tr, 1, 1))

            # --- attention -------------------------------------------------
            def norm_chain(p, Q, y_aug, act=False):
                """Emit thunks that normalize y_aug (divide rows 0:64 by the
                ones-row 64) into yT_sb[p][:, quarter Q]. Data transposes go
                through the DMA xbar (blocked 3D form); only the tiny [1,128]
                softmax-sum rows transpose on the PE (K=1, f32 precision)."""
                qlo = Q * 512
                st = {}

                def cp():
                    t = normsb.tile([65, 512], BF16, tag="ysb", name="ysb")
                    ts = normsb.tile([1, 512], F32, tag="srow", name="srow")
                    if act:  # tail: ACT is idle there
                        nc.scalar.copy(t[0:65, :], y_aug[0][0:65, :])
                        nc.scalar.copy(ts[:], y_aug[0][64:65, :])
                    else:
                        nc.vector.tensor_copy(t[0:65, :], y_aug[0][0:65, :])
                        nc.vector.tensor_copy(ts[:], y_aug[0][64:65, :])
                    st["ysb0"], st["srow0"] = t, ts

                def cp1():
                    t = normsb.tile([65, 512], BF16, tag="ysb1", name="ysb1")
                    ts = normsb.tile([1, 512], F32, tag="srow1", name="srow1")
                    if act:
                        nc.scalar.copy(t[0:65, :], y_aug[1][0:65, :])
                        nc.scalar.copy(ts[:], y_aug[1][64:65, :])
                    else:
                        nc.vector.tensor_copy(t[0:65, :], y_aug[1][0:65, :])
                        nc.vector.tensor_copy(ts[:], y_aug[1][64:65, :])
                    st["ysb1"], st["srow1"] = t, ts

                def fwd():
                    # blocked xbar transpose: [64, 512] -> [128, 4, 64]
                    for hd in range(2):
                        yn = normsb.tile(
                            [128, 4, 64], BF16, tag=f"ynat{hd}", name=f"ynat{hd}"
                        )
                        nc.sync.dma_start_transpose(
                            yn[:], st[f"ysb{hd}"][0:64, :]
                        )
                        st[f"ynat{hd}"] = yn
                    st["ynorm"] = normsb.tile(
                        [128, 4, 128], BF16, tag="ynorm", name="ynorm"
                    )

                def s_t(hd):
                    def f():
                        # transpose the four [1,128] sum rows into one PSUM
                        # tile, one reciprocal for all four
                        sps = flow.tile([128, 4], F32, tag="flow", name="flow")
                        for tb in range(4):
                            nc.tensor.transpose(
                                sps[:, tb : tb + 1],
                                st[f"srow{hd}"][:, tb * 128 : (tb + 1) * 128],
                                ident32[0:1, 0:1],
                            )
                        r = normsb.tile([128, 4], F32, tag=f"rcp{hd}",
                                        name=f"rcp{hd}")
                        nc.vector.reciprocal(r[:], sps[:])
                        st[f"r{hd}"] = r

                    return f

                def tb_step(tb):
                    def f():
                        for hd in range(2):
                            nc.vector.tensor_scalar_mul(
                                st["ynorm"][:, tb, hd * 64 : hd * 64 + 64],
                                st[f"ynat{hd}"][:, tb, :],
                                st[f"r{hd}"][:, tb : tb + 1],
                            )

                    return f

                def back(half):
                    # blocked xbar transpose per 256-col half so outproj
                    # thunks queued right behind wait on a smaller transfer
                    def f():
                        nc.sync.dma_start_transpose(
                            yT_sb[p][
                                :, qlo + half * 256 : qlo + (half + 1) * 256
                            ].rearrange("p (tb t) -> p tb t", tb=2),
                            st["ynorm"][:, 2 * half : 2 * half + 2, :].rearrange(
                                "p tb c -> p (tb c)"
                            ),
                        )

                    return f

                return [cp, cp1, fwd, s_t(0), s_t(1), tb_step(0), tb_step(1),
                        back(0), tb_step(2), tb_step(3), back(1)]

            def outproj(tb, act=False):
                def f():
                    o_sb = outp.tile([128, C], F16, tag="osb", name="osb")
                    for nch in range(2):
                        ps = flow.tile([128, 512], F32, tag="flow", name="flow")
                        for p in range(NPAIRS):
                            nc.tensor.matmul(
                                ps[:],
                                yT_sb[p][:, tb * 128 : (tb + 1) * 128],
                                wo_sb[p][:, nch * 512 : (nch + 1) * 512],
                                start=(p == 0),
                                stop=(p == NPAIRS - 1),
                            )
                        dst = o_sb[:, nch * 512 : (nch + 1) * 512]
                        if act:
                            # tail: ACT is idle after the last exp
                            nc.scalar.copy(dst, ps[:])
                        else:
                            # in-attention: ACT is exp-bound, keep it clear
                            nc.vector.tensor_copy(dst, ps[:])
                    nc.gpsimd.dma_start(out[tb], o_sb[:])

                return f

            # --- final-quarter (1,3) per-block norm+outproj ----------------
            # The last quarter's post-processing is the serial tail of the
            # whole kernel: do it per 128-col block (block b completes at AV
            # kb=12+b) with PE transposes instead of DMA-xbar ones, in two
            # stages so the cross-engine latency hides in attention iters.
            tailst = {}

            def tail_a(b, y_aug):
                st = {}
                ysb_t = normsb.tile([64, 256], F32, tag="tys", name="tys")
                srow_t = normsb.tile([1, 256], F32, tag="tsr", name="tsr")
                bl, bh = b * 128, (b + 1) * 128
                nc.vector.tensor_copy(ysb_t[:, 0:128], y_aug[0][0:64, bl:bh])
                nc.vector.tensor_copy(srow_t[:, 0:128], y_aug[0][64:65, bl:bh])
                nc.scalar.copy(ysb_t[:, 128:256], y_aug[1][0:64, bl:bh])
                nc.scalar.copy(srow_t[:, 128:256], y_aug[1][64:65, bl:bh])
                sps = flow.tile([128, 2], F32, tag="flow", name="tsps")
                for hd in range(2):
                    nc.tensor.transpose(
                        sps[:, hd : hd + 1],
                        srow_t[:, hd * 128 : (hd + 1) * 128],
                        ident32[0:1, 0:1],
                    )
                r = normsb.tile([128, 2], F32, tag="trcp", name="trcp")
                nc.vector.reciprocal(r[:], sps[:])
                ynat = flow.tile([128, 128], F32, tag="flow", name="tyn")
                for hd in range(2):
                    nc.tensor.transpose(
                        ynat[:, hd * 64 : (hd + 1) * 64],
                        ysb_t[:, hd * 128 : (hd + 1) * 128],
                        ident32[0:64, 0:64],
                    )
                st["ynat"], st["r"] = ynat, r
                tailst[b] = st

            def tail_b(b):
                st = tailst[b]
                ynat, r = st["ynat"], st["r"]
                tb = 12 + b
                q0 = 1536 + b * 128
                ynn = normsb.tile([128, 128], F32, tag="tynn", name="tynn")
                for hd in range(2):
                    nc.vector.tensor_scalar_mul(
                        ynn[:, hd * 64 : (hd + 1) * 64],
                        ynat[:, hd * 64 : (hd + 1) * 64],
                        r[:, hd : hd + 1],
                    )
                ytp = flow.tile([128, 128], F32, tag="flow", name="tytp")
                nc.tensor.transpose(ytp[:], ynn[:], ident32[:])
                nc.scalar.copy(yT_sb[1][:, q0 : q0 + 128], ytp[:])
                outproj(tb)()

            for p in range(NPAIRS):
                for Q in range(4):  # q quarters of 512
                    qlo = Q * 512
                    final = (p, Q) == (1, 3)
                    y_aug = [
                        acc.tile([128, 512], F32, tag="acc", name="acc")
                        for _ in range(2)
                    ]
                    nkb = 4 * Q + 4
                    # software pipeline: emit scores/exp for iteration i, then
                    # the av matmuls for iteration i-1, so the PE never waits
                    # on the exp of the tile it is about to consume.
                    # Each PSUM tile holds BOTH heads' score chunk (cols 0:512
                    # head A, 512:1024 head B) so one (strided) ACT exp
                    # covers them, halving ACT instruction-startup overhead.
                    # AV lags scores by TWO iterations: with lag-1 the PE
                    # waits ~0.5us per iteration on the ~1.05us ACT exp of
                    # the tile it is about to consume whenever the backlog
                    # has no filler; lag-2 decouples them entirely.
                    groups = []

                    def flush_one(p=p, qlo=qlo, nkb=nkb, y_aug=y_aug,
                                  groups=groups):
                        for (kb, hd, at, q0, wdt) in groups.pop(0):
                            nc.tensor.matmul(
                                y_aug[hd][:, q0 - qlo : q0 - qlo + wdt],
                                v_sb[p][:, kb, hd * 128 : (hd + 1) * 128],
                                at[:, hd * 512 : hd * 512 + wdt],
                                start=(kb == 0),
                                stop=(kb == nkb - 1),
                            )

                    for kb in range(nkb):
                        qs = max(kb * 128, qlo)  # global q start
                        wdt = qlo + 512 - qs
                        ps = flow.tile([128, 1024], F32, tag="flow", name="flow")
                        for hd in range(2):
                            nc.tensor.matmul(
                                ps[:, hd * 512 : hd * 512 + wdt],
                                kpad[p][hd][:, kb * 128 : (kb + 1) * 128],
                                q_sb[p][:, qs : qs + wdt],
                                start=True,
                                stop=True,
                            )
                        if len(groups) >= 2:
                            flush_one()
                        if p == 1 and Q == 2 and 4 <= kb < 8:
                            # quarter (1,0)'s outprojs run inline here: its
                            # back-transposes landed a full quarter ago, so
                            # the in-order PE queue never blocks on them
                            outproj(kb - 4)()
                        if final and kb >= 8:
                            # (1,1)'s and (1,2)'s outprojs likewise (as early
                            # backlog thunks they head-of-line-stalled the PE
                            # ~4us waiting on in-flight back-transposes)
                            outproj(kb - 4)()
                        if final and kb >= 14:
                            # block kb-14 finished accumulating in that
                            # flush (its diagonal k-block is kb-2)
                            if kb >= 15:
                                tail_b(kb - 15)
                            tail_a(kb - 14, y_aug)
                        drain_one()
                        if len(backlog) > 10:
                            drain_one()
                        if len(backlog) > 20:
                            drain_one()
                        at = attnp.tile([128, 1024], BF16, tag="attn",
                                        name="attn")
                        if wdt == 512:
                            nc.scalar.activation(
                                at[:], ps[:], mybir.ActivationFunctionType.Exp
                            )
                        else:
                            v2 = ps[:].rearrange("p (h w) -> p h w", h=2)
                            a2 = at[:].rearrange("p (h w) -> p h w", h=2)
                            nc.scalar.activation(
                                a2[:, :, 0:wdt],
                                v2[:, :, 0:wdt],
                                mybir.ActivationFunctionType.Exp,
                            )
                        if kb * 128 >= qlo:
                            # diagonal block: causal mask, both heads
                            for hd in range(2):
                                nc.vector.tensor_mul(
                                    at[:, hd * 512 : hd * 512 + 128],
                                    at[:, hd * 512 : hd * 512 + 128],
                                    tri[:],
                                )
                        groups.append(
                            [(kb, hd, at, qs, wdt) for hd in range(2)]
                        )
                    if final:
                        flush_one()  # group 14 -> block 2 complete
                        tail_a(2, y_aug)
                        tail_b(1)
                        flush_one()  # group 15 -> block 3 complete
                        tail_a(3, y_aug)
                        tail_b(2)
                        tail_b(3)
                    else:
                        while groups:
                            flush_one()
                        backlog.extend(norm_chain(p, Q, y_aug))
            drain_all()

    split_waits(nc)
    nc.finalize()
    return nc


def _get_nc():
    global _nc_cache
    if _nc_cache is None:
        _nc_cache = build_nc()
    return _nc_cache


def _prep_core_inputs(x, Wq, Wk, Wv, Wo, core):
    b, g = core // 4, core % 4
    bf = ml_dtypes.bfloat16
    xT_host = np.ascontiguousarray(x[b].T).reshape(KC, 128, T).astype(bf)

    def pack_w(Wm, scale=1.0):
        # [NPAIRS, 128, KC, 128]: lhsT chunks; columns = 2 heads' channels
        outw = np.empty((NPAIRS, 128, KC, 128), dtype=bf)
        for p in range(NPAIRS):
            h0 = 4 * g + 2 * p
            blk = Wm[h0 * D : (h0 + 2) * D, :].T * scale  # [C, 128]
            outw[p] = blk.reshape(KC, 128, 128).transpose(1, 0, 2).astype(bf)
        return outw

    scale = 1.0 / np.sqrt(D)
    wo_host = np.empty((NPAIRS, 128, C), dtype=bf)
    for p in range(NPAIRS):
        h0 = 4 * g + 2 * p
        wo_host[p] = Wo[:, h0 * D : (h0 + 2) * D].T.astype(bf)
    return {
        "xT": xT_host,
        "wq": pack_w(Wq, scale),
        "wk": pack_w(Wk),
        "wv": pack_w(Wv),
        "wo": wo_host,
    }


def _ensure_ntff_hook():
    """antenv.axon_hooks is missing from this image; shim it and register
    the ctypes NTFF profile hook so trace=True captures profiles."""
    import types

    if "antenv.axon_hooks" in sys.modules:
        return
    mod = types.ModuleType("antenv.axon_hooks")
    mod._hook = None

    def set_axon_ntff_profile_hook(h):
        mod._hook = h

    def get_axon_ntff_profile_hook():
        return mod._hook

    mod.set_axon_ntff_profile_hook = set_axon_ntff_profile_hook
    mod.get_axon_ntff_profile_hook = get_axon_ntff_profile_hook
    sys.modules["antenv.axon_hooks"] = mod
    try:
        from trn_agent_boot.trn_boot import _ntff_profile_via_ctypes

        mod._hook = _ntff_profile_via_ctypes("/opt/axon/libaxon_pjrt.so")
    except Exception as e:
        print(f"ntff hook setup failed: {e}")


def kernel(x, Wq, Wk, Wv, Wo, _trace=False, _tracedir=None):
    if _trace:
        _ensure_ntff_hook()
    x = np.asarray(x, dtype=np.float32)
    Wq, Wk, Wv, Wo = (np.asarray(w, dtype=np.float32) for w in (Wq, Wk, Wv, Wo))
    nc = _get_nc()
    in_maps = [_prep_core_inputs(x, Wq, Wk, Wv, Wo, c) for c in range(NCORES)]
    res = run_bass_kernel_spmd(
        nc, in_maps, core_ids=list(range(NCORES)), trace=_trace, tmpdir=_tracedir
    )
    out = np.zeros((NB, T, C), dtype=np.float32)
    for c in range(NCORES):
        out[c // 4] += res.results[c]["out"].astype(np.float32).reshape(T, C)
    if _trace:
        kernel._last_results = res
    return out
